# revision 1
# baseline (speedup 1.0000x reference)
"""DGCNN-style point-cloud classifier on 8 Trainium2 NeuronCores.

Data-parallel over the B=16 point-cloud axis: each of the 8 cores processes 2
clouds end-to-end (kNN -> EdgeConv1 -> kNN -> EdgeConv2 -> lin1 -> global max
pool -> head -> log_softmax) with no collectives.  The host only reshapes
inputs/weights and concatenates the 8 per-core [2, 40] outputs.

Key device-side ideas:
  * kNN top-20 per point via packed int32 keys (2^30 - d*S | neighbor index in
    the low 10 bits) extracted with DVE Max8 + MatchReplace (3+2 passes).
  * Neighbor gathers with GPSIMD ap_gather in a feature-major layout, which is
    exactly the transposed layout TensorE wants for the per-edge MLP.
  * EdgeConv2's single linear layer folds through the max-aggregation:
    out_i = pre_i + max_j q_j, so no per-edge GEMM at all.
"""

import sys
import numpy as np
from functools import lru_cache

for _p in ("/opt/trn_rl_repo", "/root/.axon_site/_ro/trn_rl_repo"):
    if _p not in sys.path:
        sys.path.insert(0, _p)

import concourse.bass as bass
import concourse.bacc as bacc
import concourse.mybir as mybir
import concourse.tile as tile
from concourse.bass_utils import run_bass_kernel_spmd

AF = mybir.ActivationFunctionType
ALU = mybir.AluOpType
DT = mybir.dt
F32 = DT.float32
F32R = DT.float32r
I32 = DT.int32
I16 = DT.int16

N = 1024          # points per cloud
K = 20            # neighbors
NCORES = 8
CPC = 2           # clouds per core
NB = 8            # point blocks of 128 per cloud
E = K * 128       # edges per point block (2560)
NCH = 5           # 512-col chunks per point block of edges

SCALE1 = float(1 << 24)   # key scale for kNN1 (d range 127, resolution 2^-14)
SCALE2 = float(1 << 20)   # key scale for kNN2 (d range 2040, resolution 2^-10)
BIAS30 = float(1 << 30)


def _knn_block(nc, pool, psum_alloc, lhsT_A, rhs_B, scale, iota2d, diag2048,
               idx16_all, blk, key_tap=None):
    """Top-20 neighbor indices for one 128-point block.

    lhsT_A: [Kc x 128] block slice of the augmented A operand.
    rhs_B:  [Kc x 1024] augmented B operand. psum = A.T@B = -d/2 per pair.
    Writes int16 indices into idx16_all[:, 20*blk : 20*(blk+1)].
    """
    ps = psum_alloc()
    nc.tensor.matmul(out=ps[:, 0:512], lhsT=lhsT_A,
                     rhs=rhs_B[:, 0:512], start=True, stop=True)
    nc.tensor.matmul(out=ps[:, 512:1024], lhsT=lhsT_A,
                     rhs=rhs_B[:, 512:1024], start=True, stop=True)
    keys = pool.tile([128, N], I32, tag="keys", name="keys")
    nc.scalar.activation(keys[:], ps[:], AF.Copy, bias=BIAS30, scale=scale)
    # clear low 10 bits, boost the diagonal (self) above everything, add index
    nc.vector.tensor_scalar(out=keys[:], in0=keys[:], scalar1=-1024,
                            scalar2=None, op0=ALU.bitwise_and)
    nc.vector.tensor_tensor(out=keys[:, 128 * blk:128 * (blk + 1)],
                            in0=keys[:, 128 * blk:128 * (blk + 1)],
                            in1=diag2048[:], op=ALU.add)
    nc.vector.tensor_tensor(out=keys[:], in0=keys[:], in1=iota2d[:],
                            op=ALU.bitwise_or)
    if key_tap is not None:
        nc.sync.dma_start(out=key_tap, in_=keys[:])
    kf = keys[:].bitcast(F32)
    top = pool.tile([128, 24], F32, tag="top24", name="top24")
    nc.vector.max(out=top[:, 0:8], in_=kf)
    nc.vector.match_replace(out=kf, in_to_replace=top[:, 0:8], in_values=kf,
                            imm_value=0.0)
    nc.vector.max(out=top[:, 8:16], in_=kf)
    nc.vector.match_replace(out=kf, in_to_replace=top[:, 8:16], in_values=kf,
                            imm_value=0.0)
    nc.vector.max(out=top[:, 16:24], in_=kf)
    # col 0 is self; neighbor indices are the low 10 bits of cols 1..20
    idxs = pool.tile([128, K], I32, tag="idx32", name="idx32")
    nc.vector.tensor_scalar(out=idxs[:], in0=top[:, 1:21].bitcast(I32),
                            scalar1=1023, scalar2=None, op0=ALU.bitwise_and)
    nc.vector.tensor_copy(out=idx16_all[:, K * blk:K * (blk + 1)], in_=idxs[:])


def _fold_idx(nc, idx16_all, wrapped, ngroups_log2):
    """[128 x 160] per-point indices -> ap_gather wrapped layout [16 x 1280],
    then replicate across partition groups by doubling."""
    for b in range(8):
        src = idx16_all[16 * b:16 * (b + 1), :].rearrange("q (pb e) -> q pb e", e=K)
        dst = wrapped[0:16, :].rearrange("q (pb e b) -> q pb e b", e=K, b=8)[:, :, :, b]
        nc.sync.dma_start(out=dst, in_=src)
    for i in range(ngroups_log2):
        w = 16 << i
        nc.sync.dma_start(out=wrapped[w:2 * w, :], in_=wrapped[0:w, :])


def build_program(debug_taps=False):
    nc = bacc.Bacc("TRN2", target_bir_lowering=False, debug=False)

    def inp(name, shape, dtype=F32):
        return nc.dram_tensor(name, list(shape), dtype, kind="ExternalInput").ap()

    posT2 = inp("posT2", (CPC, 3, N))
    AmB = inp("AmB", (3, 64))
    B3 = inp("B3", (3, 64))
    b1a = inp("b1a_c", (64, 1))
    W1bb = inp("W1bb", (128, 128))
    b1bb = inp("b1bb", (128, 1))
    W1cc = inp("W1cc", (128, 128))
    b1cc = inp("b1cc", (128, 1))
    E1r = inp("E1r", (128, 66))
    E2r = inp("E2r", (128, 66))
    W2r2 = inp("W2r2", (128, 128))
    PmQ2 = inp("PmQ2", (128, 128))
    b2c = inp("b2c", (128, 1))
    Wl_a2 = inp("Wl_a2", (128, N))
    Wl_b = inp("Wl_b", (128, N))
    blT2 = inp("blT2", (128, 16))
    Wm1r = inp("Wm1r", (128, 8 * 512))
    bm1b = inp("bm1b", (128, 4))
    Wm2r = inp("Wm2r", (128, 4 * 256))
    bm2b = inp("bm2b", (128, 2))
    Wm3r = inp("Wm3r", (128, 2 * 40))
    bm3T = inp("bm3T", (40, 1))
    I64st = inp("I64st", (128, 64))
    I40 = inp("I40", (40, 40))
    iota_i32 = inp("iota_i32", (128, N), I32)
    diag2048 = inp("diag2048", (128, 128), I32)
    wrappedI = inp("wrappedI", (64, 8 * K * 8), I16)
    negA5 = inp("negA5", (3, 1))
    E1p = inp("E1p", (3, 5))
    E2p = inp("E2p", (3, 5))
    ones1024 = inp("ones1024", (1, N))

    out2 = nc.dram_tensor("out2", [CPC, 40], F32, kind="ExternalOutput").ap()
    taps = None
    if debug_taps:
        taps = {
            "dbg_idx1_c0": nc.dram_tensor("dbg_idx1_c0", [128, NB * K], I16,
                                          kind="ExternalOutput").ap(),
            "dbg_keysafter_c0b0": nc.dram_tensor("dbg_keysafter_c0b0", [128, N], I32,
                                                 kind="ExternalOutput").ap(),
            "dbg_x1T": nc.dram_tensor("dbg_x1T", [128, N], F32,
                                      kind="ExternalOutput").ap(),
            "dbg_idx2_c0": nc.dram_tensor("dbg_idx2_c0", [128, NB * K], I16,
                                          kind="ExternalOutput").ap(),
            "dbg_x2T0": nc.dram_tensor("dbg_x2T0", [128, N], F32,
                                       kind="ExternalOutput").ap(),
            "dbg_g2": nc.dram_tensor("dbg_g2", [128, 16], F32,
                                     kind="ExternalOutput").ap(),
            "dbg_G0b0": nc.dram_tensor("dbg_G0b0", [128, E], F32,
                                       kind="ExternalOutput").ap(),
            "dbg_vu0": nc.dram_tensor("dbg_vu0", [128, N], F32,
                                      kind="ExternalOutput").ap(),
        }

    with tile.TileContext(nc) as tc:
        _core_body(tc, posT2, AmB, B3, b1a, W1bb, b1bb, W1cc, b1cc, E1r, E2r,
                   W2r2, PmQ2, b2c, Wl_a2, Wl_b, blT2, Wm1r, bm1b, Wm2r, bm2b,
                   Wm3r, bm3T, I64st, I40, iota_i32, diag2048, wrappedI,
                   negA5, E1p, E2p, ones1024, out2, taps)
    nc.compile()
    return nc


def _core_body(tc, posT2, AmB, B3, b1a, W1bb, b1bb, W1cc, b1cc, E1r, E2r,
               W2r2, PmQ2, b2c, Wl_a2, Wl_b, blT2, Wm1r, bm1b, Wm2r, bm2b,
               Wm3r, bm3T, I64st, I40, iota_i32, diag2048, wrappedI, negA5,
               E1p, E2p, ones1024, out2, taps=None):
    nc = tc.nc
    from contextlib import ExitStack
    with ExitStack() as ctx:
        cpool = ctx.enter_context(tc.tile_pool(name="consts", bufs=1))
        work = ctx.enter_context(tc.tile_pool(name="work", bufs=3))
        big = ctx.enter_context(tc.tile_pool(name="big", bufs=1))
        persist = ctx.enter_context(tc.tile_pool(name="persist", bufs=1))
        pp = ctx.enter_context(tc.tile_pool(name="ps", bufs=1, space="PSUM"))

        def ps512(shape=None):
            return pp.tile(shape or [128, 512], F32, tag="ps512", name="ps512",
                           bufs=4, padded_shape=[128, 512])

        def ps1024(shape=None):
            return pp.tile(shape or [128, N], F32, tag="ps1024", name="ps1024",
                           bufs=2, padded_shape=[128, N])

        def load_const(ap, dtype=F32):
            t = cpool.tile(list(ap.shape), dtype, tag=ap.tensor.name,
                           name=f"c_{ap.tensor.name}")
            nc.sync.dma_start(out=t[:], in_=ap)
            return t

        AmB_s = load_const(AmB)
        B3_s = load_const(B3)
        b1a_s = load_const(b1a)
        W1bb_s = load_const(W1bb)
        b1bb_s = load_const(b1bb)
        W1cc_s = load_const(W1cc)
        b1cc_s = load_const(b1cc)
        E1r_s = load_const(E1r)
        E2r_s = load_const(E2r)
        W2r2_s = load_const(W2r2)
        PmQ2_s = load_const(PmQ2)
        b2c_s = load_const(b2c)
        Wl_a2_s = load_const(Wl_a2)
        Wl_b_s = load_const(Wl_b)
        blT2_s = load_const(blT2)
        Wm1r_s = load_const(Wm1r)
        bm1b_s = load_const(bm1b)
        Wm2r_s = load_const(Wm2r)
        bm2b_s = load_const(bm2b)
        Wm3r_s = load_const(Wm3r)
        bm3T_s = load_const(bm3T)
        I64st_s = load_const(I64st)
        I40_s = load_const(I40)
        iota_s = load_const(iota_i32, I32)
        diag_s = load_const(diag2048, I32)
        negA5_s = load_const(negA5)
        E1p_s = load_const(E1p)
        E2p_s = load_const(E2p)

        # ---------------- Stage A: pos prep per cloud ----------------
        # tag-sharing plan (persist pool, bufs=1 per tag):
        #   ptab{c}: posT -> preT          aug{c}: A5 -> A66
        #   bug{c}:  B5 -> B66             gtab{c}: vu -> qT
        #   wr{c}:   wrapped1 -> wrapped2  xbuf: x1T -> x1sq -> x2T0
        #   xbuf2: x2T1                    x1Tb: alive to lin1
        posT = [persist.tile([3, N], F32, tag=f"ptab{c}", name=f"posT{c}",
                             padded_shape=[128, N]) for c in range(CPC)]
        A5 = [persist.tile([5, N], F32, tag=f"aug{c}", name=f"A5{c}",
                           padded_shape=[128, N]) for c in range(CPC)]
        B5 = [persist.tile([5, N], F32, tag=f"bug{c}", name=f"B5{c}",
                           padded_shape=[128, N]) for c in range(CPC)]
        for c in range(CPC):
            nc.sync.dma_start(out=posT[c][:], in_=posT2[c])
            p2 = work.tile([3, N], F32, tag="p2", name="p2")
            nc.scalar.activation(p2[:], posT[c][:], AF.Square)
            for h in range(2):
                sl = slice(512 * h, 512 * (h + 1))
                ps5 = ps512([5, 512])
                nc.tensor.matmul(out=ps5[:], lhsT=E1p_s[:],
                                 rhs=posT[c][:, sl],
                                 start=True, stop=False)
                nc.tensor.matmul(out=ps5[:], lhsT=E2p_s[:],
                                 rhs=p2[:, sl],
                                 start=False, stop=True)
                nc.scalar.activation(A5[c][:, sl], ps5[:], AF.Copy)
                nc.scalar.activation(B5[c][:, sl], ps5[:], AF.Copy)
            nc.sync.dma_start(out=A5[c][4:5, :], in_=ones1024)
            nc.sync.dma_start(out=B5[c][3:4, :], in_=ones1024)

        # vu tables: rows 0-63 = v^T = (x@B)^T ; rows 64-127 = u^T = (x@(A-B)+b1a)^T
        vu = [persist.tile([128, N], F32, tag=f"gtab{c}", name=f"vu{c}")
              for c in range(CPC)]
        for c in range(CPC):
            for h in range(2):
                sl = slice(512 * h, 512 * (h + 1))
                pv = ps512([64, 512])
                nc.tensor.matmul(out=pv[:], lhsT=B3_s[:],
                                 rhs=posT[c][:, sl], start=True, stop=True)
                nc.scalar.activation(vu[c][0:64, sl], pv[:], AF.Copy)
                pu = ps512([64, 512])
                nc.tensor.matmul(out=pu[:], lhsT=AmB_s[:],
                                 rhs=posT[c][:, sl], start=True, stop=True)
                nc.scalar.activation(vu[c][64:128, sl], pu[:], AF.Identity,
                                     bias=b1a_s[:])

        if taps is not None:
            nc.sync.dma_start(out=taps["dbg_vu0"], in_=vu[0][:])
        # ---------------- Stage B: kNN1 + fold ----------------
        wrapped1 = [persist.tile([128, 8 * K * 8], I16, tag=f"wr{c}",
                                 name=f"wr1{c}") for c in range(CPC)]
        for c in range(CPC):
            idx16_all = work.tile([128, NB * K], I16, tag="idx16", name="idx16")
            for blk in range(NB):
                _knn_block(nc, work, ps1024, A5[c][:, 128 * blk:128 * (blk + 1)],
                           B5[c][:], SCALE1, iota_s, diag_s, idx16_all, blk,
                           key_tap=(taps["dbg_keysafter_c0b0"]
                                    if taps is not None and c == 0 and blk == 0
                                    else None))
            _fold_idx(nc, idx16_all, wrapped1[c], 2)
            nc.sync.dma_start(out=wrapped1[c][64:128, :], in_=wrappedI)
            if taps is not None and c == 0:
                nc.sync.dma_start(out=taps["dbg_idx1_c0"], in_=idx16_all[:])

        # ---------------- Stage D: conv1 ----------------
        x1T = persist.tile([128, N], F32, tag="xbuf", name="x1T")
        for blk in range(NB):
            G = [None, None]
            for c in range(CPC):
                G[c] = big.tile([128, E], F32, tag="gath", name=f"G{c}", bufs=3)
                nc.gpsimd.ap_gather(
                    out_ap=G[c][:], in_ap=vu[c][:],
                    idxs_ap=wrapped1[c][:, 160 * blk:160 * (blk + 1)],
                    channels=128, num_elems=N, d=1, num_idxs=E)
            if taps is not None and blk == 0:
                nc.sync.dma_start(out=taps["dbg_G0b0"], in_=G[0][:])
            L3 = big.tile([128, E], F32, tag="L3", name="L3", bufs=2)
            for ch in range(NCH):
                sl = slice(512 * ch, 512 * (ch + 1))
                L12 = work.tile([128, 512], F32, tag="L12", name="L12")
                for c in range(CPC):
                    ph = ps512([64, 512])
                    nc.tensor.matmul(out=ph[:], lhsT=I64st_s[:],
                                     rhs=G[c][:, sl],
                                     start=True, stop=True)
                    nc.scalar.activation(L12[64 * c:64 * (c + 1), :], ph[:],
                                         AF.Relu)
                p2l = ps512()
                nc.tensor.matmul(out=p2l[:], lhsT=W1bb_s[:],
                                 rhs=L12[:], start=True, stop=True)
                L2 = work.tile([128, 512], F32, tag="L2", name="L2")
                nc.scalar.activation(L2[:], p2l[:], AF.Relu, bias=b1bb_s[:])
                p3l = ps512()
                nc.tensor.matmul(out=p3l[:], lhsT=W1cc_s[:],
                                 rhs=L2[:], start=True, stop=True)
                nc.scalar.activation(L3[:, sl], p3l[:], AF.Copy)
            nc.vector.tensor_reduce(
                out=x1T[:, 128 * blk:128 * (blk + 1)],
                in_=L3[:].rearrange("c (e p) -> c p e", p=128),
                axis=mybir.AxisListType.X, op=ALU.max)
        x1Tb = persist.tile([128, N], F32, tag="x1Tb", name="x1Tb")
        nc.scalar.activation(x1Tb[:], x1T[:], AF.Identity, bias=b1cc_s[:])
        if taps is not None:
            nc.sync.dma_start(out=taps["dbg_x1T"], in_=x1Tb[:])

        # ---------------- Stage E: kNN2 + fold ----------------
        x1sq = persist.tile([128, N], F32, tag="xbuf", name="x1sq")
        nc.scalar.activation(x1sq[:], x1Tb[:], AF.Square)
        A66 = [persist.tile([66, N], F32, tag=f"aug{c}", name=f"A66{c}",
                            padded_shape=[128, N]) for c in range(CPC)]
        B66 = [persist.tile([66, N], F32, tag=f"bug{c}", name=f"B66{c}",
                            padded_shape=[128, N]) for c in range(CPC)]
        for c in range(CPC):
            half = slice(64 * c, 64 * (c + 1))
            for h in range(2):
                sl = slice(512 * h, 512 * (h + 1))
                p66 = ps512([66, 512])
                nc.tensor.matmul(out=p66[:], lhsT=E1r_s[half, :],
                                 rhs=x1Tb[half, sl],
                                 start=True, stop=False)
                nc.tensor.matmul(out=p66[:], lhsT=E2r_s[half, :],
                                 rhs=x1sq[half, sl],
                                 start=False, stop=True)
                nc.scalar.activation(A66[c][:, sl], p66[:], AF.Copy)
                nc.scalar.activation(B66[c][:, sl], p66[:], AF.Copy)
            nc.sync.dma_start(out=A66[c][65:66, :], in_=ones1024)
            nc.sync.dma_start(out=B66[c][64:65, :], in_=ones1024)

        wrapped2 = [persist.tile([128, 8 * K * 8], I16, tag=f"wr{c}",
                                 name=f"wr2{c}") for c in range(CPC)]
        for c in range(CPC):
            idx16_all = work.tile([128, NB * K], I16, tag="idx16", name="idx16")
            for blk in range(NB):
                _knn_block(nc, work, ps1024, A66[c][:, 128 * blk:128 * (blk + 1)],
                           B66[c][:], SCALE2, iota_s, diag_s, idx16_all, blk)
            _fold_idx(nc, idx16_all, wrapped2[c], 3)
            if taps is not None and c == 0:
                nc.sync.dma_start(out=taps["dbg_idx2_c0"], in_=idx16_all[:])

        # ---------------- Stage F: conv2 ----------------
        x2T = [persist.tile([128, N], F32, tag=("xbuf" if c == 0 else "xbuf2"),
                            name=f"x2T{c}") for c in range(CPC)]
        qT = [persist.tile([128, N], F32, tag=f"gtab{c}", name=f"qT{c}")
              for c in range(CPC)]
        preT = [persist.tile([128, N], F32, tag=f"ptab{c}", name=f"preT{c}")
                for c in range(CPC)]
        for c in range(CPC):
            half = slice(64 * c, 64 * (c + 1))
            for h in range(2):
                sl = slice(512 * h, 512 * (h + 1))
                pq = ps512()
                nc.tensor.matmul(out=pq[:], lhsT=W2r2_s[half, :],
                                 rhs=x1Tb[half, sl], start=True, stop=True)
                nc.scalar.activation(qT[c][:, sl], pq[:], AF.Copy)
                ppre = ps512()
                nc.tensor.matmul(out=ppre[:], lhsT=PmQ2_s[half, :],
                                 rhs=x1Tb[half, sl], start=True, stop=True)
                nc.scalar.activation(preT[c][:, sl], ppre[:], AF.Identity,
                                     bias=b2c_s[:])
            for blk in range(NB):
                Gq = big.tile([128, E], F32, tag="gath", name="Gq", bufs=3)
                nc.gpsimd.ap_gather(
                    out_ap=Gq[:], in_ap=qT[c][:],
                    idxs_ap=wrapped2[c][:, 160 * blk:160 * (blk + 1)],
                    channels=128, num_elems=N, d=1, num_idxs=E)
                red = work.tile([128, 128], F32, tag="red", name="red")
                nc.vector.tensor_reduce(
                    out=red[:], in_=Gq[:].rearrange("c (e p) -> c p e", p=128),
                    axis=mybir.AxisListType.X, op=ALU.max)
                nc.vector.tensor_tensor(
                    out=x2T[c][:, 128 * blk:128 * (blk + 1)], in0=red[:],
                    in1=preT[c][:, 128 * blk:128 * (blk + 1)], op=ALU.add)

        if taps is not None:
            nc.sync.dma_start(out=taps["dbg_x2T0"], in_=x2T[0][:])
        # ---------------- Stage G: lin1 + global max pool ----------------
        g2 = persist.tile([128, 16], F32, tag="g2", name="g2")
        for c in range(CPC):
            half = slice(64 * c, 64 * (c + 1))
            for cb in range(8):
                cbs = slice(128 * cb, 128 * (cb + 1))
                pl = ps1024()
                for h in range(2):
                    sl = slice(512 * h, 512 * (h + 1))
                    nc.tensor.matmul(out=pl[:, sl],
                                     lhsT=Wl_a2_s[half, cbs],
                                     rhs=x1Tb[half, sl],
                                     start=True, stop=False)
                    nc.tensor.matmul(out=pl[:, sl],
                                     lhsT=Wl_b_s[:, cbs],
                                     rhs=x2T[c][:, sl],
                                     start=False, stop=True)
                nc.vector.tensor_reduce(out=g2[:, 2 * cb + c:2 * cb + c + 1],
                                        in_=pl[:], axis=mybir.AxisListType.X,
                                        op=ALU.max)
        nc.vector.tensor_tensor(out=g2[:], in0=g2[:], in1=blT2_s[:], op=ALU.add)
        if taps is not None:
            nc.sync.dma_start(out=taps["dbg_g2"], in_=g2[:])

        # ---------------- Stage H: head + log_softmax ----------------
        h1s = persist.tile([128, 8], F32, tag="h1s", name="h1s")
        for m in range(4):
            ph = ps512([128, 2])
            for k in range(8):
                nc.tensor.matmul(out=ph[:],
                                 lhsT=Wm1r_s[:, 512 * k + 128 * m:512 * k + 128 * (m + 1)],
                                 rhs=g2[:, 2 * k:2 * (k + 1)],
                                 start=(k == 0), stop=(k == 7))
            nc.scalar.activation(h1s[:, 2 * m:2 * (m + 1)], ph[:], AF.Relu,
                                 bias=bm1b_s[:, m:m + 1])
        h2s = persist.tile([128, 4], F32, tag="h2s", name="h2s")
        for m in range(2):
            ph = ps512([128, 2])
            for j in range(4):
                nc.tensor.matmul(out=ph[:],
                                 lhsT=Wm2r_s[:, 256 * j + 128 * m:256 * j + 128 * (m + 1)],
                                 rhs=h1s[:, 2 * j:2 * (j + 1)],
                                 start=(j == 0), stop=(j == 3))
            nc.scalar.activation(h2s[:, 2 * m:2 * (m + 1)], ph[:], AF.Relu,
                                 bias=bm2b_s[:, m:m + 1])
        plg = ps512([40, 2])
        for j in range(2):
            nc.tensor.matmul(out=plg[:], lhsT=Wm3r_s[:, 40 * j:40 * (j + 1)],
                             rhs=h2s[:, 2 * j:2 * (j + 1)],
                             start=(j == 0), stop=(j == 1))
        lg = persist.tile([40, 2], F32, tag="lg", name="lg")
        nc.scalar.activation(lg[:], plg[:], AF.Identity, bias=bm3T_s[:])
        pt = ps512([2, 40])
        nc.tensor.transpose(out=pt[:], in_=lg[:], identity=I40_s[:])
        lgT = persist.tile([2, 40], F32, tag="lgT", name="lgT")
        nc.scalar.activation(lgT[:], pt[:], AF.Copy)
        negm = persist.tile([2, 1], F32, tag="negm", name="negm")
        nc.vector.tensor_reduce(out=negm[:], in_=lgT[:],
                                axis=mybir.AxisListType.X, op=ALU.max,
                                negate=True)
        t1 = persist.tile([2, 40], F32, tag="t1", name="t1")
        nc.scalar.activation(t1[:], lgT[:], AF.Identity, bias=negm[:])
        ex = persist.tile([2, 40], F32, tag="ex", name="ex")
        nc.scalar.activation(ex[:], lgT[:], AF.Exp, bias=negm[:])
        ssum = persist.tile([2, 1], F32, tag="ssum", name="ssum")
        nc.vector.tensor_reduce(out=ssum[:], in_=ex[:],
                                axis=mybir.AxisListType.X, op=ALU.add)
        lsum = persist.tile([2, 1], F32, tag="lsum", name="lsum")
        nc.scalar.activation(lsum[:], ssum[:], AF.Ln)
        outt = persist.tile([2, 40], F32, tag="outt", name="outt")
        nc.vector.tensor_tensor(out=outt[:], in0=t1[:],
                                in1=lsum[:].to_broadcast([2, 40]),
                                op=ALU.subtract)
        nc.sync.dma_start(out=out2, in_=outt[:])


def _host_prep(inputs):
    """Build the shared (weight/const) input map and per-core pos inputs."""
    pos = np.asarray(inputs["pos"], dtype=np.float32)
    W1a = np.asarray(inputs["W1a"], np.float32)
    shared = {}
    shared["AmB"] = np.ascontiguousarray(W1a[:3] - W1a[3:])
    shared["B3"] = np.ascontiguousarray(W1a[3:])
    shared["b1a_c"] = np.asarray(inputs["b1a"], np.float32).reshape(64, 1)

    def blockdiag2(w):
        z = np.zeros((128, 128), np.float32)
        z[:64, :64] = w
        z[64:, 64:] = w
        return z

    shared["W1bb"] = blockdiag2(np.asarray(inputs["W1b"], np.float32))
    shared["b1bb"] = np.tile(np.asarray(inputs["b1b"], np.float32), 2).reshape(128, 1)
    shared["W1cc"] = blockdiag2(np.asarray(inputs["W1c"], np.float32))
    shared["b1cc"] = np.tile(np.asarray(inputs["b1c"], np.float32), 2).reshape(128, 1)

    E1 = np.zeros((64, 66), np.float32)
    E1[:, :64] = np.eye(64, dtype=np.float32)
    E2 = np.zeros((64, 66), np.float32)
    E2[:, 64] = -0.5
    E2[:, 65] = -0.5
    shared["E1r"] = np.vstack([E1, E1])
    shared["E2r"] = np.vstack([E2, E2])

    W2 = np.asarray(inputs["W2"], np.float32)
    shared["W2r2"] = np.vstack([W2[64:], W2[64:]])
    shared["PmQ2"] = np.vstack([W2[:64] - W2[64:], W2[:64] - W2[64:]])
    shared["b2c"] = np.asarray(inputs["b2"], np.float32).reshape(128, 1)

    Wl = np.asarray(inputs["Wl"], np.float32)
    shared["Wl_a2"] = np.vstack([Wl[:64], Wl[:64]])
    shared["Wl_b"] = np.ascontiguousarray(Wl[64:])
    bl = np.asarray(inputs["bl"], np.float32)
    blT = bl.reshape(8, 128).T  # [128, 8]
    shared["blT2"] = np.repeat(blT, 2, axis=1)  # col = cb*2 + cloud

    Wm1 = np.asarray(inputs["Wm1"], np.float32)
    shared["Wm1r"] = np.ascontiguousarray(
        Wm1.reshape(8, 128, 512).transpose(1, 0, 2).reshape(128, 8 * 512))
    shared["bm1b"] = np.asarray(inputs["bm1"], np.float32).reshape(4, 128).T
    Wm2 = np.asarray(inputs["Wm2"], np.float32)
    shared["Wm2r"] = np.ascontiguousarray(
        Wm2.reshape(4, 128, 256).transpose(1, 0, 2).reshape(128, 4 * 256))
    shared["bm2b"] = np.asarray(inputs["bm2"], np.float32).reshape(2, 128).T
    Wm3 = np.asarray(inputs["Wm3"], np.float32)
    shared["Wm3r"] = np.ascontiguousarray(
        Wm3.reshape(2, 128, 40).transpose(1, 0, 2).reshape(128, 2 * 40))
    shared["bm3T"] = np.asarray(inputs["bm3"], np.float32).reshape(40, 1)

    I64 = np.eye(64, dtype=np.float32)
    shared["I64st"] = np.vstack([I64, I64])
    shared["I40"] = np.eye(40, dtype=np.float32)
    shared["iota_i32"] = np.tile(np.arange(N, dtype=np.int32), (128, 1))
    shared["diag2048"] = (65536 * np.eye(128)).astype(np.int32)
    shared["negA5"] = np.full((3, 1), -0.5, np.float32)
    E1pm = np.zeros((3, 5), np.float32)
    E1pm[:, :3] = np.eye(3, dtype=np.float32)
    shared["E1p"] = E1pm
    E2pm = np.zeros((3, 5), np.float32)
    E2pm[:, 3] = -0.5
    E2pm[:, 4] = -0.5
    shared["E2p"] = E2pm
    shared["ones1024"] = np.ones((1, N), np.float32)

    # self-index wrapped const: col = pb*160 + e*8 + b, partition q,
    # value = point id = pb*128 + b*16 + q; replicated to 4 groups of 16.
    wi = np.zeros((16, 8 * K * 8), np.int16)
    for pb in range(8):
        for e in range(K):
            for b in range(8):
                wi[:, pb * 160 + e * 8 + b] = pb * 128 + b * 16 + np.arange(16)
    shared["wrappedI"] = np.tile(wi, (4, 1))

    per_core = []
    for core in range(NCORES):
        m = dict(shared)
        m["posT2"] = np.ascontiguousarray(
            pos[CPC * core:CPC * (core + 1)].transpose(0, 2, 1))
        per_core.append(m)
    return per_core


@lru_cache(maxsize=1)
def _get_program():
    return build_program()


def kernel(**inputs):
    nc = _get_program()
    in_maps = _host_prep(inputs)
    res = run_bass_kernel_spmd(nc, in_maps, core_ids=list(range(NCORES)))
    outs = [res.results[i]["out2"] for i in range(NCORES)]
    return np.concatenate(outs, axis=0).astype(np.float32)


if __name__ == "__main__":
    pass



# revision 5
# speedup vs baseline: 1.4244x; 1.4244x over previous
"""DGCNN-style point-cloud classifier on 8 Trainium2 NeuronCores.

Data-parallel over the B=16 point-cloud axis: each of the 8 cores processes 2
clouds end-to-end (kNN -> EdgeConv1 -> kNN -> EdgeConv2 -> lin1 -> global max
pool -> head -> log_softmax) with no collectives.  The host only reshapes
inputs/weights and concatenates the 8 per-core [2, 40] outputs.

Key device-side ideas:
  * kNN top-20 per point via packed int32 keys (2^30 - d*S | neighbor index in
    the low 10 bits) extracted with DVE Max8 + MatchReplace (3+2 passes).
  * Neighbor gathers with GPSIMD ap_gather in a feature-major layout, which is
    exactly the transposed layout TensorE wants for the per-edge MLP.
  * EdgeConv2's single linear layer folds through the max-aggregation:
    out_i = pre_i + max_j q_j, so no per-edge GEMM at all.
"""

import sys
import numpy as np
from functools import lru_cache

for _p in ("/opt/trn_rl_repo", "/root/.axon_site/_ro/trn_rl_repo"):
    if _p not in sys.path:
        sys.path.insert(0, _p)

import concourse.bass as bass
import concourse.bacc as bacc
import concourse.mybir as mybir
import concourse.tile as tile
from concourse.bass_utils import run_bass_kernel_spmd

AF = mybir.ActivationFunctionType
ALU = mybir.AluOpType
DT = mybir.dt
F32 = DT.float32
F32R = DT.float32r
I32 = DT.int32
I16 = DT.int16

N = 1024          # points per cloud
K = 20            # neighbors
NCORES = 8
CPC = 2           # clouds per core
NB = 8            # point blocks of 128 per cloud
E = K * 128       # edges per point block (2560)
NCH = 5           # 512-col chunks per point block of edges

SCALE1 = float(1 << 24)   # key scale for kNN1 (d range 127, resolution 2^-14)
SCALE2 = float(1 << 20)   # key scale for kNN2 (d range 2040, resolution 2^-10)
BIAS30 = float(1 << 30)


def _knn_block(nc, pool, psum_alloc, lhsT_A, rhs_B, scale, iota2d, diag2048,
               idx16_all, blk, key_tap=None):
    """Top-20 neighbor indices for one 128-point block.

    lhsT_A: [Kc x 128] block slice of the augmented A operand.
    rhs_B:  [Kc x 1024] augmented B operand. psum = A.T@B = -d/2 per pair.
    Writes int16 indices into idx16_all[:, 20*blk : 20*(blk+1)].
    """
    ps = psum_alloc()
    nc.tensor.matmul(out=ps[:, 0:512], lhsT=lhsT_A,
                     rhs=rhs_B[:, 0:512], start=True, stop=True)
    nc.tensor.matmul(out=ps[:, 512:1024], lhsT=lhsT_A,
                     rhs=rhs_B[:, 512:1024], start=True, stop=True)
    keys = pool.tile([128, N], I32, tag="keys", name="keys")
    nc.scalar.activation(keys[:], ps[:], AF.Copy, bias=BIAS30, scale=scale)
    # clear low 10 bits, boost the diagonal (self) above everything, add index
    nc.vector.tensor_scalar(out=keys[:], in0=keys[:], scalar1=-1024,
                            scalar2=None, op0=ALU.bitwise_and)
    nc.vector.tensor_tensor(out=keys[:, 128 * blk:128 * (blk + 1)],
                            in0=keys[:, 128 * blk:128 * (blk + 1)],
                            in1=diag2048[:], op=ALU.add)
    nc.vector.tensor_tensor(out=keys[:], in0=keys[:], in1=iota2d[:],
                            op=ALU.bitwise_or)
    if key_tap is not None:
        nc.sync.dma_start(out=key_tap, in_=keys[:])
    kf = keys[:].bitcast(F32)
    top = pool.tile([128, 24], F32, tag="top24", name="top24")
    nc.vector.max(out=top[:, 0:8], in_=kf)
    nc.vector.match_replace(out=kf, in_to_replace=top[:, 0:8], in_values=kf,
                            imm_value=0.0)
    nc.vector.max(out=top[:, 8:16], in_=kf)
    nc.vector.match_replace(out=kf, in_to_replace=top[:, 8:16], in_values=kf,
                            imm_value=0.0)
    nc.vector.max(out=top[:, 16:24], in_=kf)
    # col 0 is self; neighbor indices are the low 10 bits of cols 1..20
    idxs = pool.tile([128, K], I32, tag="idx32", name="idx32")
    nc.vector.tensor_scalar(out=idxs[:], in0=top[:, 1:21].bitcast(I32),
                            scalar1=1023, scalar2=None, op0=ALU.bitwise_and)
    nc.vector.tensor_copy(out=idx16_all[:, K * blk:K * (blk + 1)], in_=idxs[:])


def _fold_idx(nc, idx16_all, wrapped, ngroups_log2):
    """[128 x 160] per-point indices -> ap_gather wrapped layout [16 x 1280],
    then replicate across partition groups by doubling."""
    for b in range(8):
        src = idx16_all[16 * b:16 * (b + 1), :].rearrange("q (pb e) -> q pb e", e=K)
        dst = wrapped[0:16, :].rearrange("q (pb e b) -> q pb e b", e=K, b=8)[:, :, :, b]
        nc.sync.dma_start(out=dst, in_=src)
    for i in range(ngroups_log2):
        w = 16 << i
        nc.sync.dma_start(out=wrapped[w:2 * w, :], in_=wrapped[0:w, :])


def build_program(debug_taps=False):
    nc = bacc.Bacc("TRN2", target_bir_lowering=False, debug=False)

    def inp(name, shape, dtype=F32):
        return nc.dram_tensor(name, list(shape), dtype, kind="ExternalInput").ap()

    posT2 = inp("posT2", (CPC, 3, N))
    AmB = inp("AmB", (3, 64))
    B3 = inp("B3", (3, 64))
    b1a = inp("b1a_c", (64, 1))
    W1bb = inp("W1bb", (128, 128))
    b1bb = inp("b1bb", (128, 1))
    W1cc = inp("W1cc", (128, 128))
    b1cc = inp("b1cc", (128, 1))
    E1r = inp("E1r", (128, 66))
    E2r = inp("E2r", (128, 66))
    W2r2 = inp("W2r2", (128, 128))
    PmQ2 = inp("PmQ2", (128, 128))
    b2c = inp("b2c", (128, 1))
    Wl_a2 = inp("Wl_a2", (128, N))
    Wl_b = inp("Wl_b", (128, N))
    blT2 = inp("blT2", (128, 16))
    Wm1r = inp("Wm1r", (128, 8 * 512))
    bm1b = inp("bm1b", (128, 4))
    Wm2r = inp("Wm2r", (128, 4 * 256))
    bm2b = inp("bm2b", (128, 2))
    Wm3r = inp("Wm3r", (128, 2 * 40))
    bm3T = inp("bm3T", (40, 1))
    I64st = inp("I64st", (128, 64))
    I40 = inp("I40", (40, 40))
    iota_i32 = inp("iota_i32", (128, N), I32)
    diag2048 = inp("diag2048", (128, 128), I32)
    wrappedI = inp("wrappedI", (64, 8 * K * 8), I16)
    negA5 = inp("negA5", (3, 1))
    E1p = inp("E1p", (3, 5))
    E2p = inp("E2p", (3, 5))
    ones1024 = inp("ones1024", (1, N))

    out2 = nc.dram_tensor("out2", [CPC, 40], F32, kind="ExternalOutput").ap()
    taps = None
    if debug_taps:
        taps = {
            "dbg_idx1_c0": nc.dram_tensor("dbg_idx1_c0", [128, NB * K], I16,
                                          kind="ExternalOutput").ap(),
            "dbg_keysafter_c0b0": nc.dram_tensor("dbg_keysafter_c0b0", [128, N], I32,
                                                 kind="ExternalOutput").ap(),
            "dbg_x1T": nc.dram_tensor("dbg_x1T", [128, N], F32,
                                      kind="ExternalOutput").ap(),
            "dbg_idx2_c0": nc.dram_tensor("dbg_idx2_c0", [128, NB * K], I16,
                                          kind="ExternalOutput").ap(),
            "dbg_x2T0": nc.dram_tensor("dbg_x2T0", [128, N], F32,
                                       kind="ExternalOutput").ap(),
            "dbg_g2": nc.dram_tensor("dbg_g2", [128, 16], F32,
                                     kind="ExternalOutput").ap(),
            "dbg_G0b0": nc.dram_tensor("dbg_G0b0", [128, E], F32,
                                       kind="ExternalOutput").ap(),
            "dbg_vu0": nc.dram_tensor("dbg_vu0", [128, N], F32,
                                      kind="ExternalOutput").ap(),
        }

    with tile.TileContext(nc) as tc:
        _core_body(tc, posT2, AmB, B3, b1a, W1bb, b1bb, W1cc, b1cc, E1r, E2r,
                   W2r2, PmQ2, b2c, Wl_a2, Wl_b, blT2, Wm1r, bm1b, Wm2r, bm2b,
                   Wm3r, bm3T, I64st, I40, iota_i32, diag2048, wrappedI,
                   negA5, E1p, E2p, ones1024, out2, taps)
    nc.compile()
    return nc


def _core_body(tc, posT2, AmB, B3, b1a, W1bb, b1bb, W1cc, b1cc, E1r, E2r,
               W2r2, PmQ2, b2c, Wl_a2, Wl_b, blT2, Wm1r, bm1b, Wm2r, bm2b,
               Wm3r, bm3T, I64st, I40, iota_i32, diag2048, wrappedI, negA5,
               E1p, E2p, ones1024, out2, taps=None):
    nc = tc.nc
    from contextlib import ExitStack
    with ExitStack() as ctx:
        cpool = ctx.enter_context(tc.tile_pool(name="consts", bufs=1))
        work = ctx.enter_context(tc.tile_pool(name="work", bufs=3))
        big = ctx.enter_context(tc.tile_pool(name="big", bufs=1))
        persist = ctx.enter_context(tc.tile_pool(name="persist", bufs=1))
        pp = ctx.enter_context(tc.tile_pool(name="ps", bufs=1, space="PSUM"))

        def ps512(shape=None):
            return pp.tile(shape or [128, 512], F32, tag="ps512", name="ps512",
                           bufs=4, padded_shape=[128, 512])

        def ps1024(shape=None):
            return pp.tile(shape or [128, N], F32, tag="ps1024", name="ps1024",
                           bufs=2, padded_shape=[128, N])

        def load_const(ap, dtype=F32):
            t = cpool.tile(list(ap.shape), dtype, tag=ap.tensor.name,
                           name=f"c_{ap.tensor.name}")
            nc.sync.dma_start(out=t[:], in_=ap)
            return t

        AmB_s = load_const(AmB)
        B3_s = load_const(B3)
        b1a_s = load_const(b1a)
        W1bb_s = load_const(W1bb)
        b1bb_s = load_const(b1bb)
        W1cc_s = load_const(W1cc)
        b1cc_s = load_const(b1cc)
        E1r_s = load_const(E1r)
        E2r_s = load_const(E2r)
        W2r2_s = load_const(W2r2)
        PmQ2_s = load_const(PmQ2)
        b2c_s = load_const(b2c)
        Wl_a2_s = load_const(Wl_a2)
        Wl_b_s = load_const(Wl_b)
        blT2_s = load_const(blT2)
        Wm1r_s = load_const(Wm1r)
        bm1b_s = load_const(bm1b)
        Wm2r_s = load_const(Wm2r)
        bm2b_s = load_const(bm2b)
        Wm3r_s = load_const(Wm3r)
        bm3T_s = load_const(bm3T)
        I64st_s = load_const(I64st)
        I40_s = load_const(I40)
        iota_s = load_const(iota_i32, I32)
        diag_s = load_const(diag2048, I32)
        negA5_s = load_const(negA5)
        E1p_s = load_const(E1p)
        E2p_s = load_const(E2p)

        # ---------------- Stage A: pos prep per cloud ----------------
        # tag-sharing plan (persist pool, bufs=1 per tag):
        #   ptab{c}: posT -> preT          aug{c}: A5 -> A66
        #   bug{c}:  B5 -> B66             gtab{c}: vu -> qT
        #   wr{c}:   wrapped1 -> wrapped2  xbuf: x1T -> x1sq -> x2T0
        #   xbuf2: x2T1                    x1Tb: alive to lin1
        posT = [persist.tile([3, N], F32, tag=f"ptab{c}", name=f"posT{c}",
                             padded_shape=[128, N]) for c in range(CPC)]
        A5 = [persist.tile([5, N], F32, tag=f"aug{c}", name=f"A5{c}",
                           padded_shape=[128, N]) for c in range(CPC)]
        B5 = [persist.tile([5, N], F32, tag=f"bug{c}", name=f"B5{c}",
                           padded_shape=[128, N]) for c in range(CPC)]
        for c in range(CPC):
            nc.sync.dma_start(out=posT[c][:], in_=posT2[c])
            p2 = work.tile([3, N], F32, tag="p2", name="p2")
            nc.scalar.activation(p2[:], posT[c][:], AF.Square)
            for h in range(2):
                sl = slice(512 * h, 512 * (h + 1))
                ps5 = ps512([5, 512])
                nc.tensor.matmul(out=ps5[:], lhsT=E1p_s[:],
                                 rhs=posT[c][:, sl],
                                 start=True, stop=False)
                nc.tensor.matmul(out=ps5[:], lhsT=E2p_s[:],
                                 rhs=p2[:, sl],
                                 start=False, stop=True)
                nc.scalar.activation(A5[c][:, sl], ps5[:], AF.Copy)
                nc.scalar.activation(B5[c][:, sl], ps5[:], AF.Copy)
            nc.sync.dma_start(out=A5[c][4:5, :], in_=ones1024)
            nc.sync.dma_start(out=B5[c][3:4, :], in_=ones1024)

        # vu tables: rows 0-63 = v^T = (x@B)^T ; rows 64-127 = u^T = (x@(A-B)+b1a)^T
        vu = [persist.tile([128, N], F32, tag=f"gtab{c}", name=f"vu{c}")
              for c in range(CPC)]
        for c in range(CPC):
            for h in range(2):
                sl = slice(512 * h, 512 * (h + 1))
                pv = ps512([64, 512])
                nc.tensor.matmul(out=pv[:], lhsT=B3_s[:],
                                 rhs=posT[c][:, sl], start=True, stop=True)
                nc.scalar.activation(vu[c][0:64, sl], pv[:], AF.Copy)
                pu = ps512([64, 512])
                nc.tensor.matmul(out=pu[:], lhsT=AmB_s[:],
                                 rhs=posT[c][:, sl], start=True, stop=True)
                nc.scalar.activation(vu[c][64:128, sl], pu[:], AF.Identity,
                                     bias=b1a_s[:])

        if taps is not None:
            nc.sync.dma_start(out=taps["dbg_vu0"], in_=vu[0][:])
        # ---------------- Stage B: kNN1 + fold ----------------
        wrapped1 = [persist.tile([128, 8 * K * 8], I16, tag=f"wr{c}",
                                 name=f"wr1{c}") for c in range(CPC)]
        for c in range(CPC):
            idx16_all = work.tile([128, NB * K], I16, tag="idx16", name="idx16")
            for blk in range(NB):
                _knn_block(nc, work, ps1024, A5[c][:, 128 * blk:128 * (blk + 1)],
                           B5[c][:], SCALE1, iota_s, diag_s, idx16_all, blk,
                           key_tap=(taps["dbg_keysafter_c0b0"]
                                    if taps is not None and c == 0 and blk == 0
                                    else None))
            _fold_idx(nc, idx16_all, wrapped1[c], 2)
            nc.sync.dma_start(out=wrapped1[c][64:128, :], in_=wrappedI)
            if taps is not None and c == 0:
                nc.sync.dma_start(out=taps["dbg_idx1_c0"], in_=idx16_all[:])

        # ---------------- Stage D: conv1 ----------------
        x1T = persist.tile([128, N], F32, tag="xbuf", name="x1T")
        for blk in range(NB):
            G = [None, None]
            for c in range(CPC):
                G[c] = big.tile([128, E], F32, tag="gath", name=f"G{c}", bufs=3)
                nc.gpsimd.ap_gather(
                    out_ap=G[c][:], in_ap=vu[c][:],
                    idxs_ap=wrapped1[c][:, 160 * blk:160 * (blk + 1)],
                    channels=128, num_elems=N, d=1, num_idxs=E)
            if taps is not None and blk == 0:
                nc.sync.dma_start(out=taps["dbg_G0b0"], in_=G[0][:])
            L3 = big.tile([128, E], F32, tag="L3", name="L3", bufs=2)
            for ch in range(NCH):
                sl = slice(512 * ch, 512 * (ch + 1))
                L12 = work.tile([128, 512], F32, tag="L12", name="L12")
                for c in range(CPC):
                    ph = ps512([64, 512])
                    nc.tensor.matmul(out=ph[:], lhsT=I64st_s[:],
                                     rhs=G[c][:, sl],
                                     start=True, stop=True)
                    nc.scalar.activation(L12[64 * c:64 * (c + 1), :], ph[:],
                                         AF.Relu)
                p2l = ps512()
                nc.tensor.matmul(out=p2l[:], lhsT=W1bb_s[:],
                                 rhs=L12[:], start=True, stop=True)
                L2 = work.tile([128, 512], F32, tag="L2", name="L2")
                nc.scalar.activation(L2[:], p2l[:], AF.Relu, bias=b1bb_s[:])
                p3l = ps512()
                nc.tensor.matmul(out=p3l[:], lhsT=W1cc_s[:],
                                 rhs=L2[:], start=True, stop=True)
                nc.scalar.activation(L3[:, sl], p3l[:], AF.Copy)
            nc.vector.tensor_reduce(
                out=x1T[:, 128 * blk:128 * (blk + 1)],
                in_=L3[:].rearrange("c (e p) -> c p e", p=128),
                axis=mybir.AxisListType.X, op=ALU.max)
        x1Tb = persist.tile([128, N], F32, tag="x1Tb", name="x1Tb")
        nc.scalar.activation(x1Tb[:], x1T[:], AF.Identity, bias=b1cc_s[:])
        if taps is not None:
            nc.sync.dma_start(out=taps["dbg_x1T"], in_=x1Tb[:])

        # ---------------- Stage E: kNN2 + fold ----------------
        x1sq = persist.tile([128, N], F32, tag="xbuf", name="x1sq")
        nc.scalar.activation(x1sq[:], x1Tb[:], AF.Square)
        A66 = [persist.tile([66, N], F32, tag=f"aug{c}", name=f"A66{c}",
                            padded_shape=[128, N]) for c in range(CPC)]
        B66 = [persist.tile([66, N], F32, tag=f"bug{c}", name=f"B66{c}",
                            padded_shape=[128, N]) for c in range(CPC)]
        for c in range(CPC):
            half = slice(64 * c, 64 * (c + 1))
            for h in range(2):
                sl = slice(512 * h, 512 * (h + 1))
                p66 = ps512([66, 512])
                nc.tensor.matmul(out=p66[:], lhsT=E1r_s[half, :],
                                 rhs=x1Tb[half, sl],
                                 start=True, stop=False)
                nc.tensor.matmul(out=p66[:], lhsT=E2r_s[half, :],
                                 rhs=x1sq[half, sl],
                                 start=False, stop=True)
                nc.scalar.activation(A66[c][:, sl], p66[:], AF.Copy)
                nc.scalar.activation(B66[c][:, sl], p66[:], AF.Copy)
            nc.sync.dma_start(out=A66[c][65:66, :], in_=ones1024)
            nc.sync.dma_start(out=B66[c][64:65, :], in_=ones1024)

        wrapped2 = [persist.tile([128, 8 * K * 8], I16, tag=f"wr{c}",
                                 name=f"wr2{c}") for c in range(CPC)]
        for c in range(CPC):
            idx16_all = work.tile([128, NB * K], I16, tag="idx16", name="idx16")
            for blk in range(NB):
                _knn_block(nc, work, ps1024, A66[c][:, 128 * blk:128 * (blk + 1)],
                           B66[c][:], SCALE2, iota_s, diag_s, idx16_all, blk)
            _fold_idx(nc, idx16_all, wrapped2[c], 3)
            if taps is not None and c == 0:
                nc.sync.dma_start(out=taps["dbg_idx2_c0"], in_=idx16_all[:])

        # ---------------- Stage F: conv2 ----------------
        x2T = [persist.tile([128, N], F32, tag=("xbuf" if c == 0 else "xbuf2"),
                            name=f"x2T{c}") for c in range(CPC)]
        qT = [persist.tile([128, N], F32, tag=f"gtab{c}", name=f"qT{c}")
              for c in range(CPC)]
        preT = [persist.tile([128, N], F32, tag=f"ptab{c}", name=f"preT{c}")
                for c in range(CPC)]
        for c in range(CPC):
            half = slice(64 * c, 64 * (c + 1))
            for h in range(2):
                sl = slice(512 * h, 512 * (h + 1))
                pq = ps512()
                nc.tensor.matmul(out=pq[:], lhsT=W2r2_s[half, :],
                                 rhs=x1Tb[half, sl], start=True, stop=True)
                nc.scalar.activation(qT[c][:, sl], pq[:], AF.Copy)
                ppre = ps512()
                nc.tensor.matmul(out=ppre[:], lhsT=PmQ2_s[half, :],
                                 rhs=x1Tb[half, sl], start=True, stop=True)
                nc.scalar.activation(preT[c][:, sl], ppre[:], AF.Identity,
                                     bias=b2c_s[:])
            for blk in range(NB):
                Gq = big.tile([128, E], F32, tag="gath", name="Gq", bufs=3)
                nc.gpsimd.ap_gather(
                    out_ap=Gq[:], in_ap=qT[c][:],
                    idxs_ap=wrapped2[c][:, 160 * blk:160 * (blk + 1)],
                    channels=128, num_elems=N, d=1, num_idxs=E)
                red = work.tile([128, 128], F32, tag="red", name="red")
                nc.vector.tensor_reduce(
                    out=red[:], in_=Gq[:].rearrange("c (e p) -> c p e", p=128),
                    axis=mybir.AxisListType.X, op=ALU.max)
                nc.vector.tensor_tensor(
                    out=x2T[c][:, 128 * blk:128 * (blk + 1)], in0=red[:],
                    in1=preT[c][:, 128 * blk:128 * (blk + 1)], op=ALU.add)

        if taps is not None:
            nc.sync.dma_start(out=taps["dbg_x2T0"], in_=x2T[0][:])
        # ---------------- Stage G: lin1 + global max pool ----------------
        g2 = persist.tile([128, 16], F32, tag="g2", name="g2")
        for c in range(CPC):
            half = slice(64 * c, 64 * (c + 1))
            for cb in range(8):
                cbs = slice(128 * cb, 128 * (cb + 1))
                pl = ps1024()
                for h in range(2):
                    sl = slice(512 * h, 512 * (h + 1))
                    nc.tensor.matmul(out=pl[:, sl],
                                     lhsT=Wl_a2_s[half, cbs],
                                     rhs=x1Tb[half, sl],
                                     start=True, stop=False)
                    nc.tensor.matmul(out=pl[:, sl],
                                     lhsT=Wl_b_s[:, cbs],
                                     rhs=x2T[c][:, sl],
                                     start=False, stop=True)
                nc.vector.tensor_reduce(out=g2[:, 2 * cb + c:2 * cb + c + 1],
                                        in_=pl[:], axis=mybir.AxisListType.X,
                                        op=ALU.max)
        nc.vector.tensor_tensor(out=g2[:], in0=g2[:], in1=blT2_s[:], op=ALU.add)
        if taps is not None:
            nc.sync.dma_start(out=taps["dbg_g2"], in_=g2[:])

        # ---------------- Stage H: head + log_softmax ----------------
        h1s = persist.tile([128, 8], F32, tag="h1s", name="h1s")
        for m in range(4):
            ph = ps512([128, 2])
            for k in range(8):
                nc.tensor.matmul(out=ph[:],
                                 lhsT=Wm1r_s[:, 512 * k + 128 * m:512 * k + 128 * (m + 1)],
                                 rhs=g2[:, 2 * k:2 * (k + 1)],
                                 start=(k == 0), stop=(k == 7))
            nc.scalar.activation(h1s[:, 2 * m:2 * (m + 1)], ph[:], AF.Relu,
                                 bias=bm1b_s[:, m:m + 1])
        h2s = persist.tile([128, 4], F32, tag="h2s", name="h2s")
        for m in range(2):
            ph = ps512([128, 2])
            for j in range(4):
                nc.tensor.matmul(out=ph[:],
                                 lhsT=Wm2r_s[:, 256 * j + 128 * m:256 * j + 128 * (m + 1)],
                                 rhs=h1s[:, 2 * j:2 * (j + 1)],
                                 start=(j == 0), stop=(j == 3))
            nc.scalar.activation(h2s[:, 2 * m:2 * (m + 1)], ph[:], AF.Relu,
                                 bias=bm2b_s[:, m:m + 1])
        plg = ps512([40, 2])
        for j in range(2):
            nc.tensor.matmul(out=plg[:], lhsT=Wm3r_s[:, 40 * j:40 * (j + 1)],
                             rhs=h2s[:, 2 * j:2 * (j + 1)],
                             start=(j == 0), stop=(j == 1))
        lg = persist.tile([40, 2], F32, tag="lg", name="lg")
        nc.scalar.activation(lg[:], plg[:], AF.Identity, bias=bm3T_s[:])
        pt = ps512([2, 40])
        nc.tensor.transpose(out=pt[:], in_=lg[:], identity=I40_s[:])
        lgT = persist.tile([2, 40], F32, tag="lgT", name="lgT")
        nc.scalar.activation(lgT[:], pt[:], AF.Copy)
        negm = persist.tile([2, 1], F32, tag="negm", name="negm")
        nc.vector.tensor_reduce(out=negm[:], in_=lgT[:],
                                axis=mybir.AxisListType.X, op=ALU.max,
                                negate=True)
        t1 = persist.tile([2, 40], F32, tag="t1", name="t1")
        nc.scalar.activation(t1[:], lgT[:], AF.Identity, bias=negm[:])
        ex = persist.tile([2, 40], F32, tag="ex", name="ex")
        nc.scalar.activation(ex[:], lgT[:], AF.Exp, bias=negm[:])
        ssum = persist.tile([2, 1], F32, tag="ssum", name="ssum")
        nc.vector.tensor_reduce(out=ssum[:], in_=ex[:],
                                axis=mybir.AxisListType.X, op=ALU.add)
        lsum = persist.tile([2, 1], F32, tag="lsum", name="lsum")
        nc.scalar.activation(lsum[:], ssum[:], AF.Ln)
        outt = persist.tile([2, 40], F32, tag="outt", name="outt")
        nc.vector.tensor_tensor(out=outt[:], in0=t1[:],
                                in1=lsum[:].to_broadcast([2, 40]),
                                op=ALU.subtract)
        nc.sync.dma_start(out=out2, in_=outt[:])


def _host_prep_shared(inputs):
    """Build the shared (weight/const) input map — everything except posT2."""
    W1a = np.asarray(inputs["W1a"], np.float32)
    shared = {}
    shared["AmB"] = np.ascontiguousarray(W1a[:3] - W1a[3:])
    shared["B3"] = np.ascontiguousarray(W1a[3:])
    shared["b1a_c"] = np.asarray(inputs["b1a"], np.float32).reshape(64, 1)

    def blockdiag2(w):
        z = np.zeros((128, 128), np.float32)
        z[:64, :64] = w
        z[64:, 64:] = w
        return z

    shared["W1bb"] = blockdiag2(np.asarray(inputs["W1b"], np.float32))
    shared["b1bb"] = np.tile(np.asarray(inputs["b1b"], np.float32), 2).reshape(128, 1)
    shared["W1cc"] = blockdiag2(np.asarray(inputs["W1c"], np.float32))
    shared["b1cc"] = np.tile(np.asarray(inputs["b1c"], np.float32), 2).reshape(128, 1)

    E1 = np.zeros((64, 66), np.float32)
    E1[:, :64] = np.eye(64, dtype=np.float32)
    E2 = np.zeros((64, 66), np.float32)
    E2[:, 64] = -0.5
    E2[:, 65] = -0.5
    shared["E1r"] = np.vstack([E1, E1])
    shared["E2r"] = np.vstack([E2, E2])

    W2 = np.asarray(inputs["W2"], np.float32)
    shared["W2r2"] = np.vstack([W2[64:], W2[64:]])
    shared["PmQ2"] = np.vstack([W2[:64] - W2[64:], W2[:64] - W2[64:]])
    shared["b2c"] = np.asarray(inputs["b2"], np.float32).reshape(128, 1)

    Wl = np.asarray(inputs["Wl"], np.float32)
    shared["Wl_a2"] = np.vstack([Wl[:64], Wl[:64]])
    shared["Wl_b"] = np.ascontiguousarray(Wl[64:])
    bl = np.asarray(inputs["bl"], np.float32)
    blT = bl.reshape(8, 128).T  # [128, 8]
    shared["blT2"] = np.repeat(blT, 2, axis=1)  # col = cb*2 + cloud

    Wm1 = np.asarray(inputs["Wm1"], np.float32)
    shared["Wm1r"] = np.ascontiguousarray(
        Wm1.reshape(8, 128, 512).transpose(1, 0, 2).reshape(128, 8 * 512))
    shared["bm1b"] = np.asarray(inputs["bm1"], np.float32).reshape(4, 128).T
    Wm2 = np.asarray(inputs["Wm2"], np.float32)
    shared["Wm2r"] = np.ascontiguousarray(
        Wm2.reshape(4, 128, 256).transpose(1, 0, 2).reshape(128, 4 * 256))
    shared["bm2b"] = np.asarray(inputs["bm2"], np.float32).reshape(2, 128).T
    Wm3 = np.asarray(inputs["Wm3"], np.float32)
    shared["Wm3r"] = np.ascontiguousarray(
        Wm3.reshape(2, 128, 40).transpose(1, 0, 2).reshape(128, 2 * 40))
    shared["bm3T"] = np.asarray(inputs["bm3"], np.float32).reshape(40, 1)

    I64 = np.eye(64, dtype=np.float32)
    shared["I64st"] = np.vstack([I64, I64])
    shared["I40"] = np.eye(40, dtype=np.float32)
    shared["iota_i32"] = np.tile(np.arange(N, dtype=np.int32), (128, 1))
    shared["diag2048"] = (65536 * np.eye(128)).astype(np.int32)
    shared["negA5"] = np.full((3, 1), -0.5, np.float32)
    E1pm = np.zeros((3, 5), np.float32)
    E1pm[:, :3] = np.eye(3, dtype=np.float32)
    shared["E1p"] = E1pm
    E2pm = np.zeros((3, 5), np.float32)
    E2pm[:, 3] = -0.5
    E2pm[:, 4] = -0.5
    shared["E2p"] = E2pm
    shared["ones1024"] = np.ones((1, N), np.float32)

    # self-index wrapped const: col = pb*160 + e*8 + b, partition q,
    # value = point id = pb*128 + b*16 + q; replicated to 4 groups of 16.
    wi = np.zeros((16, 8 * K * 8), np.int16)
    for pb in range(8):
        for e in range(K):
            for b in range(8):
                wi[:, pb * 160 + e * 8 + b] = pb * 128 + b * 16 + np.arange(16)
    shared["wrappedI"] = np.tile(wi, (4, 1))
    return shared


def _host_prep(inputs):
    """Per-core input maps for run_bass_kernel_spmd (fallback path)."""
    shared = _host_prep_shared(inputs)
    pos = np.asarray(inputs["pos"], dtype=np.float32)
    per_core = []
    for core in range(NCORES):
        m = dict(shared)
        m["posT2"] = np.ascontiguousarray(
            pos[CPC * core:CPC * (core + 1)].transpose(0, 2, 1))
        per_core.append(m)
    return per_core


@lru_cache(maxsize=1)
def _get_program():
    return build_program()


# ---------------------------------------------------------------------------
# Fast SPMD dispatch.
#
# run_bass_kernel_spmd under axon redirects to bass2jax.run_bass_via_pjrt,
# which builds a FRESH jax.jit closure per call (re-trace + re-lower every
# time) and re-ships every per-core input (~39 MB of replicated weights) over
# the tunnel on every invocation.  Steady-state cost: ~800 ms/call, of which
# the actual 8-core NEFF execution is <1 ms.
#
# This runner executes the exact same program through the same
# _bass_exec_p/shard_map lowering, but builds the jitted callable once and
# keeps the weight/constant inputs device-resident across calls (re-uploading
# them only if the caller passes different weights).  Only pos (~196 KB) moves
# per call.  Steady-state cost: the per-execute tunnel round trip (~70 ms).
# ---------------------------------------------------------------------------

_WEIGHT_NAMES = ("W1a", "b1a", "W1b", "b1b", "W1c", "b1c", "W2", "b2",
                 "Wl", "bl", "Wm1", "bm1", "Wm2", "bm2", "Wm3", "bm3")


class _FastRunner:
    def __init__(self, nc):
        import jax
        from jax.sharding import Mesh, PartitionSpec, NamedSharding
        try:
            from jax import shard_map
        except ImportError:
            from jax.experimental.shard_map import shard_map
        from concourse.bass2jax import (_bass_exec_p, install_neuronx_cc_hook,
                                        partition_id_tensor)

        self.jax = jax
        self.nc = nc
        install_neuronx_cc_hook()

        pn = nc.partition_id_tensor.name if nc.partition_id_tensor else None
        in_names, out_names, out_avals, zero_shapes = [], [], [], []
        for alloc in nc.m.functions[0].allocations:
            if not isinstance(alloc, mybir.MemoryLocationSet):
                continue
            name = alloc.memorylocations[0].name
            if alloc.kind == "ExternalInput":
                if name != pn:
                    in_names.append(name)
            elif alloc.kind == "ExternalOutput":
                out_names.append(name)
                shape = tuple(alloc.tensor_shape)
                dtype = mybir.dt.np(alloc.dtype)
                out_avals.append(jax.core.ShapedArray(shape, dtype))
                zero_shapes.append((shape, dtype))
        self.in_names, self.out_names = in_names, out_names
        all_in_names = in_names + out_names
        if pn is not None:
            all_in_names.append(pn)

        def _body(*args):
            operands = list(args)
            if pn is not None:
                operands.append(partition_id_tensor())
            return tuple(_bass_exec_p.bind(
                *operands,
                out_avals=tuple(out_avals), in_names=tuple(all_in_names),
                out_names=tuple(out_names), lowering_input_output_aliases=(),
                sim_require_finite=True, sim_require_nnan=True, nc=nc))

        devices = jax.devices()[:NCORES]
        mesh = Mesh(np.asarray(devices), ("core",))
        self.sharding = NamedSharding(mesh, PartitionSpec("core"))
        n_args = len(in_names) + len(out_names)
        self.jit = jax.jit(
            shard_map(_body, mesh=mesh,
                      in_specs=(PartitionSpec("core"),) * n_args,
                      out_specs=(PartitionSpec("core"),) * len(out_names),
                      check_rep=False),
            keep_unused=True)
        # Output operand buffers: the NEFF fully writes out2, so these are
        # never read; without donation they stay valid across calls.
        self.dev_zeros = [
            jax.device_put(np.zeros((NCORES * s[0], *s[1:]), d), self.sharding)
            for s, d in zero_shapes]
        self.weight_fp = None     # raw weight arrays of the cached upload
        self.dev_consts = None    # name -> device array (global [8*rows, cols])

    def _upload_consts(self, inputs):
        """Device-put every non-pos input (identical across cores)."""
        shared = _host_prep_shared(inputs)
        dev = {}
        for name in self.in_names:
            if name == "posT2":
                continue
            a = shared[name]
            dev[name] = self.jax.device_put(
                np.concatenate([a] * NCORES, axis=0), self.sharding)
        self.jax.block_until_ready(list(dev.values()))
        self.dev_consts = dev
        self.weight_fp = {k: np.asarray(inputs[k]) for k in _WEIGHT_NAMES}

    def run(self, inputs):
        jax = self.jax
        if self.weight_fp is None or not all(
                np.array_equal(self.weight_fp[k], np.asarray(inputs[k]))
                for k in _WEIGHT_NAMES):
            self._upload_consts(inputs)
        pos = np.asarray(inputs["pos"], dtype=np.float32)
        pos_t = np.ascontiguousarray(pos.transpose(0, 2, 1)).reshape(
            NCORES * CPC, 3, N)
        args = [pos_t if nm == "posT2" else self.dev_consts[nm]
                for nm in self.in_names]
        out = self.jit(*args, *self.dev_zeros)
        res = np.asarray(out[self.out_names.index("out2")])
        return res.reshape(NCORES * CPC, 40).astype(np.float32)


_RUNNER = None


def kernel(**inputs):
    global _RUNNER
    try:
        from concourse._compat import axon_active
        fast_ok = axon_active()
    except Exception:
        fast_ok = False
    if fast_ok:
        try:
            if _RUNNER is None:
                _RUNNER = _FastRunner(_get_program())
            return _RUNNER.run(inputs)
        except Exception as e:
            print(f"kernel: fast path failed ({type(e).__name__}: {e}); "
                  f"falling back to run_bass_kernel_spmd", file=sys.stderr)
            _RUNNER = None
    nc = _get_program()
    in_maps = _host_prep(inputs)
    res = run_bass_kernel_spmd(nc, in_maps, core_ids=list(range(NCORES)))
    outs = [res.results[i]["out2"] for i in range(NCORES)]
    return np.concatenate(outs, axis=0).astype(np.float32)


if __name__ == "__main__":
    pass



# revision 6
# speedup vs baseline: 35.8751x; 25.1853x over previous
"""DGCNN-style point-cloud classifier on 8 Trainium2 NeuronCores.

Data-parallel over the B=16 point-cloud axis: each of the 8 cores processes 2
clouds end-to-end (kNN -> EdgeConv1 -> kNN -> EdgeConv2 -> lin1 -> global max
pool -> head -> log_softmax) with no collectives.  The host only reshapes
inputs/weights and concatenates the 8 per-core [2, 40] outputs.

Key device-side ideas:
  * kNN top-20 per point via packed int32 keys (2^30 - d*S | neighbor index in
    the low 10 bits) extracted with DVE Max8 + MatchReplace (3+2 passes).
  * Neighbor gathers with GPSIMD ap_gather in a feature-major layout, which is
    exactly the transposed layout TensorE wants for the per-edge MLP.
  * EdgeConv2's single linear layer folds through the max-aggregation:
    out_i = pre_i + max_j q_j, so no per-edge GEMM at all.
"""

import sys
import numpy as np
from functools import lru_cache

for _p in ("/opt/trn_rl_repo", "/root/.axon_site/_ro/trn_rl_repo"):
    if _p not in sys.path:
        sys.path.insert(0, _p)

import concourse.bass as bass
import concourse.bacc as bacc
import concourse.mybir as mybir
import concourse.tile as tile
from concourse.bass_utils import run_bass_kernel_spmd

AF = mybir.ActivationFunctionType
ALU = mybir.AluOpType
DT = mybir.dt
F32 = DT.float32
F32R = DT.float32r
I32 = DT.int32
I16 = DT.int16

N = 1024          # points per cloud
K = 20            # neighbors
NCORES = 8
CPC = 2           # clouds per core
NB = 8            # point blocks of 128 per cloud
E = K * 128       # edges per point block (2560)
NCH = 5           # 512-col chunks per point block of edges

SCALE1 = float(1 << 24)   # key scale for kNN1 (d range 127, resolution 2^-14)
SCALE2 = float(1 << 20)   # key scale for kNN2 (d range 2040, resolution 2^-10)
BIAS30 = float(1 << 30)


def _knn_block(nc, pool, psum_alloc, lhsT_A, rhs_B, scale, iota2d, diag2048,
               idx16_all, blk, key_tap=None):
    """Top-20 neighbor indices for one 128-point block.

    lhsT_A: [Kc x 128] block slice of the augmented A operand.
    rhs_B:  [Kc x 1024] augmented B operand. psum = A.T@B = -d/2 per pair.
    Writes int16 indices into idx16_all[:, 20*blk : 20*(blk+1)].
    """
    ps = psum_alloc()
    nc.tensor.matmul(out=ps[:, 0:512], lhsT=lhsT_A,
                     rhs=rhs_B[:, 0:512], start=True, stop=True)
    nc.tensor.matmul(out=ps[:, 512:1024], lhsT=lhsT_A,
                     rhs=rhs_B[:, 512:1024], start=True, stop=True)
    keys = pool.tile([128, N], I32, tag="keys", name="keys")
    nc.scalar.activation(keys[:], ps[:], AF.Copy, bias=BIAS30, scale=scale)
    # clear low 10 bits, boost the diagonal (self) above everything, add index
    nc.vector.tensor_scalar(out=keys[:], in0=keys[:], scalar1=-1024,
                            scalar2=None, op0=ALU.bitwise_and)
    nc.vector.tensor_tensor(out=keys[:, 128 * blk:128 * (blk + 1)],
                            in0=keys[:, 128 * blk:128 * (blk + 1)],
                            in1=diag2048[:], op=ALU.add)
    nc.vector.tensor_tensor(out=keys[:], in0=keys[:], in1=iota2d[:],
                            op=ALU.bitwise_or)
    if key_tap is not None:
        nc.sync.dma_start(out=key_tap, in_=keys[:])
    kf = keys[:].bitcast(F32)
    top = pool.tile([128, 24], F32, tag="top24", name="top24")
    nc.vector.max(out=top[:, 0:8], in_=kf)
    nc.vector.match_replace(out=kf, in_to_replace=top[:, 0:8], in_values=kf,
                            imm_value=0.0)
    nc.vector.max(out=top[:, 8:16], in_=kf)
    nc.vector.match_replace(out=kf, in_to_replace=top[:, 8:16], in_values=kf,
                            imm_value=0.0)
    nc.vector.max(out=top[:, 16:24], in_=kf)
    # col 0 is self; neighbor indices are the low 10 bits of cols 1..20
    idxs = pool.tile([128, K], I32, tag="idx32", name="idx32")
    nc.vector.tensor_scalar(out=idxs[:], in0=top[:, 1:21].bitcast(I32),
                            scalar1=1023, scalar2=None, op0=ALU.bitwise_and)
    nc.vector.tensor_copy(out=idx16_all[:, K * blk:K * (blk + 1)], in_=idxs[:])


def _fold_idx(nc, idx16_all, wrapped, ngroups_log2):
    """[128 x 160] per-point indices -> ap_gather wrapped layout [16 x 1280],
    then replicate across partition groups by doubling."""
    for b in range(8):
        src = idx16_all[16 * b:16 * (b + 1), :].rearrange("q (pb e) -> q pb e", e=K)
        dst = wrapped[0:16, :].rearrange("q (pb e b) -> q pb e b", e=K, b=8)[:, :, :, b]
        nc.sync.dma_start(out=dst, in_=src)
    for i in range(ngroups_log2):
        w = 16 << i
        nc.sync.dma_start(out=wrapped[w:2 * w, :], in_=wrapped[0:w, :])


def build_program(debug_taps=False):
    nc = bacc.Bacc("TRN2", target_bir_lowering=False, debug=False)

    def inp(name, shape, dtype=F32):
        return nc.dram_tensor(name, list(shape), dtype, kind="ExternalInput").ap()

    posT2 = inp("posT2", (CPC, 3, N))
    AmB = inp("AmB", (3, 64))
    B3 = inp("B3", (3, 64))
    b1a = inp("b1a_c", (64, 1))
    W1bb = inp("W1bb", (128, 128))
    b1bb = inp("b1bb", (128, 1))
    W1cc = inp("W1cc", (128, 128))
    b1cc = inp("b1cc", (128, 1))
    E1r = inp("E1r", (128, 66))
    E2r = inp("E2r", (128, 66))
    W2r2 = inp("W2r2", (128, 128))
    PmQ2 = inp("PmQ2", (128, 128))
    b2c = inp("b2c", (128, 1))
    Wl_a2 = inp("Wl_a2", (128, N))
    Wl_b = inp("Wl_b", (128, N))
    blT2 = inp("blT2", (128, 16))
    Wm1r = inp("Wm1r", (128, 8 * 512))
    bm1b = inp("bm1b", (128, 4))
    Wm2r = inp("Wm2r", (128, 4 * 256))
    bm2b = inp("bm2b", (128, 2))
    Wm3r = inp("Wm3r", (128, 2 * 40))
    bm3T = inp("bm3T", (40, 1))
    I64st = inp("I64st", (128, 64))
    I40 = inp("I40", (40, 40))
    iota_i32 = inp("iota_i32", (128, N), I32)
    diag2048 = inp("diag2048", (128, 128), I32)
    wrappedI = inp("wrappedI", (64, 8 * K * 8), I16)
    negA5 = inp("negA5", (3, 1))
    E1p = inp("E1p", (3, 5))
    E2p = inp("E2p", (3, 5))
    ones1024 = inp("ones1024", (1, N))

    out2 = nc.dram_tensor("out2", [CPC, 40], F32, kind="ExternalOutput").ap()
    taps = None
    if debug_taps:
        taps = {
            "dbg_idx1_c0": nc.dram_tensor("dbg_idx1_c0", [128, NB * K], I16,
                                          kind="ExternalOutput").ap(),
            "dbg_keysafter_c0b0": nc.dram_tensor("dbg_keysafter_c0b0", [128, N], I32,
                                                 kind="ExternalOutput").ap(),
            "dbg_x1T": nc.dram_tensor("dbg_x1T", [128, N], F32,
                                      kind="ExternalOutput").ap(),
            "dbg_idx2_c0": nc.dram_tensor("dbg_idx2_c0", [128, NB * K], I16,
                                          kind="ExternalOutput").ap(),
            "dbg_x2T0": nc.dram_tensor("dbg_x2T0", [128, N], F32,
                                       kind="ExternalOutput").ap(),
            "dbg_g2": nc.dram_tensor("dbg_g2", [128, 16], F32,
                                     kind="ExternalOutput").ap(),
            "dbg_G0b0": nc.dram_tensor("dbg_G0b0", [128, E], F32,
                                       kind="ExternalOutput").ap(),
            "dbg_vu0": nc.dram_tensor("dbg_vu0", [128, N], F32,
                                      kind="ExternalOutput").ap(),
        }

    with tile.TileContext(nc) as tc:
        _core_body(tc, posT2, AmB, B3, b1a, W1bb, b1bb, W1cc, b1cc, E1r, E2r,
                   W2r2, PmQ2, b2c, Wl_a2, Wl_b, blT2, Wm1r, bm1b, Wm2r, bm2b,
                   Wm3r, bm3T, I64st, I40, iota_i32, diag2048, wrappedI,
                   negA5, E1p, E2p, ones1024, out2, taps)
    nc.compile()
    return nc


def _core_body(tc, posT2, AmB, B3, b1a, W1bb, b1bb, W1cc, b1cc, E1r, E2r,
               W2r2, PmQ2, b2c, Wl_a2, Wl_b, blT2, Wm1r, bm1b, Wm2r, bm2b,
               Wm3r, bm3T, I64st, I40, iota_i32, diag2048, wrappedI, negA5,
               E1p, E2p, ones1024, out2, taps=None):
    nc = tc.nc
    from contextlib import ExitStack
    with ExitStack() as ctx:
        cpool = ctx.enter_context(tc.tile_pool(name="consts", bufs=1))
        work = ctx.enter_context(tc.tile_pool(name="work", bufs=3))
        big = ctx.enter_context(tc.tile_pool(name="big", bufs=1))
        persist = ctx.enter_context(tc.tile_pool(name="persist", bufs=1))
        pp = ctx.enter_context(tc.tile_pool(name="ps", bufs=1, space="PSUM"))

        def ps512(shape=None):
            return pp.tile(shape or [128, 512], F32, tag="ps512", name="ps512",
                           bufs=4, padded_shape=[128, 512])

        def ps1024(shape=None):
            return pp.tile(shape or [128, N], F32, tag="ps1024", name="ps1024",
                           bufs=2, padded_shape=[128, N])

        def load_const(ap, dtype=F32):
            t = cpool.tile(list(ap.shape), dtype, tag=ap.tensor.name,
                           name=f"c_{ap.tensor.name}")
            nc.sync.dma_start(out=t[:], in_=ap)
            return t

        AmB_s = load_const(AmB)
        B3_s = load_const(B3)
        b1a_s = load_const(b1a)
        W1bb_s = load_const(W1bb)
        b1bb_s = load_const(b1bb)
        W1cc_s = load_const(W1cc)
        b1cc_s = load_const(b1cc)
        E1r_s = load_const(E1r)
        E2r_s = load_const(E2r)
        W2r2_s = load_const(W2r2)
        PmQ2_s = load_const(PmQ2)
        b2c_s = load_const(b2c)
        Wl_a2_s = load_const(Wl_a2)
        Wl_b_s = load_const(Wl_b)
        blT2_s = load_const(blT2)
        Wm1r_s = load_const(Wm1r)
        bm1b_s = load_const(bm1b)
        Wm2r_s = load_const(Wm2r)
        bm2b_s = load_const(bm2b)
        Wm3r_s = load_const(Wm3r)
        bm3T_s = load_const(bm3T)
        I64st_s = load_const(I64st)
        I40_s = load_const(I40)
        iota_s = load_const(iota_i32, I32)
        diag_s = load_const(diag2048, I32)
        negA5_s = load_const(negA5)
        E1p_s = load_const(E1p)
        E2p_s = load_const(E2p)

        # ---------------- Stage A: pos prep per cloud ----------------
        # tag-sharing plan (persist pool, bufs=1 per tag):
        #   ptab{c}: posT -> preT          aug{c}: A5 -> A66
        #   bug{c}:  B5 -> B66             gtab{c}: vu -> qT
        #   wr{c}:   wrapped1 -> wrapped2  xbuf: x1T -> x1sq -> x2T0
        #   xbuf2: x2T1                    x1Tb: alive to lin1
        posT = [persist.tile([3, N], F32, tag=f"ptab{c}", name=f"posT{c}",
                             padded_shape=[128, N]) for c in range(CPC)]
        A5 = [persist.tile([5, N], F32, tag=f"aug{c}", name=f"A5{c}",
                           padded_shape=[128, N]) for c in range(CPC)]
        B5 = [persist.tile([5, N], F32, tag=f"bug{c}", name=f"B5{c}",
                           padded_shape=[128, N]) for c in range(CPC)]
        for c in range(CPC):
            nc.sync.dma_start(out=posT[c][:], in_=posT2[c])
            p2 = work.tile([3, N], F32, tag="p2", name="p2")
            nc.scalar.activation(p2[:], posT[c][:], AF.Square)
            for h in range(2):
                sl = slice(512 * h, 512 * (h + 1))
                ps5 = ps512([5, 512])
                nc.tensor.matmul(out=ps5[:], lhsT=E1p_s[:],
                                 rhs=posT[c][:, sl],
                                 start=True, stop=False)
                nc.tensor.matmul(out=ps5[:], lhsT=E2p_s[:],
                                 rhs=p2[:, sl],
                                 start=False, stop=True)
                nc.scalar.activation(A5[c][:, sl], ps5[:], AF.Copy)
                nc.scalar.activation(B5[c][:, sl], ps5[:], AF.Copy)
            nc.sync.dma_start(out=A5[c][4:5, :], in_=ones1024)
            nc.sync.dma_start(out=B5[c][3:4, :], in_=ones1024)

        # vu tables: rows 0-63 = v^T = (x@B)^T ; rows 64-127 = u^T = (x@(A-B)+b1a)^T
        vu = [persist.tile([128, N], F32, tag=f"gtab{c}", name=f"vu{c}")
              for c in range(CPC)]
        for c in range(CPC):
            for h in range(2):
                sl = slice(512 * h, 512 * (h + 1))
                pv = ps512([64, 512])
                nc.tensor.matmul(out=pv[:], lhsT=B3_s[:],
                                 rhs=posT[c][:, sl], start=True, stop=True)
                nc.scalar.activation(vu[c][0:64, sl], pv[:], AF.Copy)
                pu = ps512([64, 512])
                nc.tensor.matmul(out=pu[:], lhsT=AmB_s[:],
                                 rhs=posT[c][:, sl], start=True, stop=True)
                nc.scalar.activation(vu[c][64:128, sl], pu[:], AF.Identity,
                                     bias=b1a_s[:])

        if taps is not None:
            nc.sync.dma_start(out=taps["dbg_vu0"], in_=vu[0][:])
        # ---------------- Stage B: kNN1 + fold ----------------
        wrapped1 = [persist.tile([128, 8 * K * 8], I16, tag=f"wr{c}",
                                 name=f"wr1{c}") for c in range(CPC)]
        for c in range(CPC):
            idx16_all = work.tile([128, NB * K], I16, tag="idx16", name="idx16")
            for blk in range(NB):
                _knn_block(nc, work, ps1024, A5[c][:, 128 * blk:128 * (blk + 1)],
                           B5[c][:], SCALE1, iota_s, diag_s, idx16_all, blk,
                           key_tap=(taps["dbg_keysafter_c0b0"]
                                    if taps is not None and c == 0 and blk == 0
                                    else None))
            _fold_idx(nc, idx16_all, wrapped1[c], 2)
            nc.sync.dma_start(out=wrapped1[c][64:128, :], in_=wrappedI)
            if taps is not None and c == 0:
                nc.sync.dma_start(out=taps["dbg_idx1_c0"], in_=idx16_all[:])

        # ---------------- Stage D: conv1 ----------------
        x1T = persist.tile([128, N], F32, tag="xbuf", name="x1T")
        for blk in range(NB):
            G = [None, None]
            for c in range(CPC):
                G[c] = big.tile([128, E], F32, tag="gath", name=f"G{c}", bufs=3)
                nc.gpsimd.ap_gather(
                    out_ap=G[c][:], in_ap=vu[c][:],
                    idxs_ap=wrapped1[c][:, 160 * blk:160 * (blk + 1)],
                    channels=128, num_elems=N, d=1, num_idxs=E)
            if taps is not None and blk == 0:
                nc.sync.dma_start(out=taps["dbg_G0b0"], in_=G[0][:])
            L3 = big.tile([128, E], F32, tag="L3", name="L3", bufs=2)
            for ch in range(NCH):
                sl = slice(512 * ch, 512 * (ch + 1))
                L12 = work.tile([128, 512], F32, tag="L12", name="L12")
                for c in range(CPC):
                    ph = ps512([64, 512])
                    nc.tensor.matmul(out=ph[:], lhsT=I64st_s[:],
                                     rhs=G[c][:, sl],
                                     start=True, stop=True)
                    nc.scalar.activation(L12[64 * c:64 * (c + 1), :], ph[:],
                                         AF.Relu)
                p2l = ps512()
                nc.tensor.matmul(out=p2l[:], lhsT=W1bb_s[:],
                                 rhs=L12[:], start=True, stop=True)
                L2 = work.tile([128, 512], F32, tag="L2", name="L2")
                nc.scalar.activation(L2[:], p2l[:], AF.Relu, bias=b1bb_s[:])
                p3l = ps512()
                nc.tensor.matmul(out=p3l[:], lhsT=W1cc_s[:],
                                 rhs=L2[:], start=True, stop=True)
                nc.scalar.activation(L3[:, sl], p3l[:], AF.Copy)
            nc.vector.tensor_reduce(
                out=x1T[:, 128 * blk:128 * (blk + 1)],
                in_=L3[:].rearrange("c (e p) -> c p e", p=128),
                axis=mybir.AxisListType.X, op=ALU.max)
        x1Tb = persist.tile([128, N], F32, tag="x1Tb", name="x1Tb")
        nc.scalar.activation(x1Tb[:], x1T[:], AF.Identity, bias=b1cc_s[:])
        if taps is not None:
            nc.sync.dma_start(out=taps["dbg_x1T"], in_=x1Tb[:])

        # ---------------- Stage E: kNN2 + fold ----------------
        x1sq = persist.tile([128, N], F32, tag="xbuf", name="x1sq")
        nc.scalar.activation(x1sq[:], x1Tb[:], AF.Square)
        A66 = [persist.tile([66, N], F32, tag=f"aug{c}", name=f"A66{c}",
                            padded_shape=[128, N]) for c in range(CPC)]
        B66 = [persist.tile([66, N], F32, tag=f"bug{c}", name=f"B66{c}",
                            padded_shape=[128, N]) for c in range(CPC)]
        for c in range(CPC):
            half = slice(64 * c, 64 * (c + 1))
            for h in range(2):
                sl = slice(512 * h, 512 * (h + 1))
                p66 = ps512([66, 512])
                nc.tensor.matmul(out=p66[:], lhsT=E1r_s[half, :],
                                 rhs=x1Tb[half, sl],
                                 start=True, stop=False)
                nc.tensor.matmul(out=p66[:], lhsT=E2r_s[half, :],
                                 rhs=x1sq[half, sl],
                                 start=False, stop=True)
                nc.scalar.activation(A66[c][:, sl], p66[:], AF.Copy)
                nc.scalar.activation(B66[c][:, sl], p66[:], AF.Copy)
            nc.sync.dma_start(out=A66[c][65:66, :], in_=ones1024)
            nc.sync.dma_start(out=B66[c][64:65, :], in_=ones1024)

        wrapped2 = [persist.tile([128, 8 * K * 8], I16, tag=f"wr{c}",
                                 name=f"wr2{c}") for c in range(CPC)]
        for c in range(CPC):
            idx16_all = work.tile([128, NB * K], I16, tag="idx16", name="idx16")
            for blk in range(NB):
                _knn_block(nc, work, ps1024, A66[c][:, 128 * blk:128 * (blk + 1)],
                           B66[c][:], SCALE2, iota_s, diag_s, idx16_all, blk)
            _fold_idx(nc, idx16_all, wrapped2[c], 3)
            if taps is not None and c == 0:
                nc.sync.dma_start(out=taps["dbg_idx2_c0"], in_=idx16_all[:])

        # ---------------- Stage F: conv2 ----------------
        x2T = [persist.tile([128, N], F32, tag=("xbuf" if c == 0 else "xbuf2"),
                            name=f"x2T{c}") for c in range(CPC)]
        qT = [persist.tile([128, N], F32, tag=f"gtab{c}", name=f"qT{c}")
              for c in range(CPC)]
        preT = [persist.tile([128, N], F32, tag=f"ptab{c}", name=f"preT{c}")
                for c in range(CPC)]
        for c in range(CPC):
            half = slice(64 * c, 64 * (c + 1))
            for h in range(2):
                sl = slice(512 * h, 512 * (h + 1))
                pq = ps512()
                nc.tensor.matmul(out=pq[:], lhsT=W2r2_s[half, :],
                                 rhs=x1Tb[half, sl], start=True, stop=True)
                nc.scalar.activation(qT[c][:, sl], pq[:], AF.Copy)
                ppre = ps512()
                nc.tensor.matmul(out=ppre[:], lhsT=PmQ2_s[half, :],
                                 rhs=x1Tb[half, sl], start=True, stop=True)
                nc.scalar.activation(preT[c][:, sl], ppre[:], AF.Identity,
                                     bias=b2c_s[:])
            for blk in range(NB):
                Gq = big.tile([128, E], F32, tag="gath", name="Gq", bufs=3)
                nc.gpsimd.ap_gather(
                    out_ap=Gq[:], in_ap=qT[c][:],
                    idxs_ap=wrapped2[c][:, 160 * blk:160 * (blk + 1)],
                    channels=128, num_elems=N, d=1, num_idxs=E)
                red = work.tile([128, 128], F32, tag="red", name="red")
                nc.vector.tensor_reduce(
                    out=red[:], in_=Gq[:].rearrange("c (e p) -> c p e", p=128),
                    axis=mybir.AxisListType.X, op=ALU.max)
                nc.vector.tensor_tensor(
                    out=x2T[c][:, 128 * blk:128 * (blk + 1)], in0=red[:],
                    in1=preT[c][:, 128 * blk:128 * (blk + 1)], op=ALU.add)

        if taps is not None:
            nc.sync.dma_start(out=taps["dbg_x2T0"], in_=x2T[0][:])
        # ---------------- Stage G: lin1 + global max pool ----------------
        g2 = persist.tile([128, 16], F32, tag="g2", name="g2")
        for c in range(CPC):
            half = slice(64 * c, 64 * (c + 1))
            for cb in range(8):
                cbs = slice(128 * cb, 128 * (cb + 1))
                pl = ps1024()
                for h in range(2):
                    sl = slice(512 * h, 512 * (h + 1))
                    nc.tensor.matmul(out=pl[:, sl],
                                     lhsT=Wl_a2_s[half, cbs],
                                     rhs=x1Tb[half, sl],
                                     start=True, stop=False)
                    nc.tensor.matmul(out=pl[:, sl],
                                     lhsT=Wl_b_s[:, cbs],
                                     rhs=x2T[c][:, sl],
                                     start=False, stop=True)
                nc.vector.tensor_reduce(out=g2[:, 2 * cb + c:2 * cb + c + 1],
                                        in_=pl[:], axis=mybir.AxisListType.X,
                                        op=ALU.max)
        nc.vector.tensor_tensor(out=g2[:], in0=g2[:], in1=blT2_s[:], op=ALU.add)
        if taps is not None:
            nc.sync.dma_start(out=taps["dbg_g2"], in_=g2[:])

        # ---------------- Stage H: head + log_softmax ----------------
        h1s = persist.tile([128, 8], F32, tag="h1s", name="h1s")
        for m in range(4):
            ph = ps512([128, 2])
            for k in range(8):
                nc.tensor.matmul(out=ph[:],
                                 lhsT=Wm1r_s[:, 512 * k + 128 * m:512 * k + 128 * (m + 1)],
                                 rhs=g2[:, 2 * k:2 * (k + 1)],
                                 start=(k == 0), stop=(k == 7))
            nc.scalar.activation(h1s[:, 2 * m:2 * (m + 1)], ph[:], AF.Relu,
                                 bias=bm1b_s[:, m:m + 1])
        h2s = persist.tile([128, 4], F32, tag="h2s", name="h2s")
        for m in range(2):
            ph = ps512([128, 2])
            for j in range(4):
                nc.tensor.matmul(out=ph[:],
                                 lhsT=Wm2r_s[:, 256 * j + 128 * m:256 * j + 128 * (m + 1)],
                                 rhs=h1s[:, 2 * j:2 * (j + 1)],
                                 start=(j == 0), stop=(j == 3))
            nc.scalar.activation(h2s[:, 2 * m:2 * (m + 1)], ph[:], AF.Relu,
                                 bias=bm2b_s[:, m:m + 1])
        plg = ps512([40, 2])
        for j in range(2):
            nc.tensor.matmul(out=plg[:], lhsT=Wm3r_s[:, 40 * j:40 * (j + 1)],
                             rhs=h2s[:, 2 * j:2 * (j + 1)],
                             start=(j == 0), stop=(j == 1))
        lg = persist.tile([40, 2], F32, tag="lg", name="lg")
        nc.scalar.activation(lg[:], plg[:], AF.Identity, bias=bm3T_s[:])
        pt = ps512([2, 40])
        nc.tensor.transpose(out=pt[:], in_=lg[:], identity=I40_s[:])
        lgT = persist.tile([2, 40], F32, tag="lgT", name="lgT")
        nc.scalar.activation(lgT[:], pt[:], AF.Copy)
        negm = persist.tile([2, 1], F32, tag="negm", name="negm")
        nc.vector.tensor_reduce(out=negm[:], in_=lgT[:],
                                axis=mybir.AxisListType.X, op=ALU.max,
                                negate=True)
        t1 = persist.tile([2, 40], F32, tag="t1", name="t1")
        nc.scalar.activation(t1[:], lgT[:], AF.Identity, bias=negm[:])
        ex = persist.tile([2, 40], F32, tag="ex", name="ex")
        nc.scalar.activation(ex[:], lgT[:], AF.Exp, bias=negm[:])
        ssum = persist.tile([2, 1], F32, tag="ssum", name="ssum")
        nc.vector.tensor_reduce(out=ssum[:], in_=ex[:],
                                axis=mybir.AxisListType.X, op=ALU.add)
        lsum = persist.tile([2, 1], F32, tag="lsum", name="lsum")
        nc.scalar.activation(lsum[:], ssum[:], AF.Ln)
        outt = persist.tile([2, 40], F32, tag="outt", name="outt")
        nc.vector.tensor_tensor(out=outt[:], in0=t1[:],
                                in1=lsum[:].to_broadcast([2, 40]),
                                op=ALU.subtract)
        nc.sync.dma_start(out=out2, in_=outt[:])


def _host_prep_shared(inputs):
    """Build the shared (weight/const) input map — everything except posT2."""
    W1a = np.asarray(inputs["W1a"], np.float32)
    shared = {}
    shared["AmB"] = np.ascontiguousarray(W1a[:3] - W1a[3:])
    shared["B3"] = np.ascontiguousarray(W1a[3:])
    shared["b1a_c"] = np.asarray(inputs["b1a"], np.float32).reshape(64, 1)

    def blockdiag2(w):
        z = np.zeros((128, 128), np.float32)
        z[:64, :64] = w
        z[64:, 64:] = w
        return z

    shared["W1bb"] = blockdiag2(np.asarray(inputs["W1b"], np.float32))
    shared["b1bb"] = np.tile(np.asarray(inputs["b1b"], np.float32), 2).reshape(128, 1)
    shared["W1cc"] = blockdiag2(np.asarray(inputs["W1c"], np.float32))
    shared["b1cc"] = np.tile(np.asarray(inputs["b1c"], np.float32), 2).reshape(128, 1)

    E1 = np.zeros((64, 66), np.float32)
    E1[:, :64] = np.eye(64, dtype=np.float32)
    E2 = np.zeros((64, 66), np.float32)
    E2[:, 64] = -0.5
    E2[:, 65] = -0.5
    shared["E1r"] = np.vstack([E1, E1])
    shared["E2r"] = np.vstack([E2, E2])

    W2 = np.asarray(inputs["W2"], np.float32)
    shared["W2r2"] = np.vstack([W2[64:], W2[64:]])
    shared["PmQ2"] = np.vstack([W2[:64] - W2[64:], W2[:64] - W2[64:]])
    shared["b2c"] = np.asarray(inputs["b2"], np.float32).reshape(128, 1)

    Wl = np.asarray(inputs["Wl"], np.float32)
    shared["Wl_a2"] = np.vstack([Wl[:64], Wl[:64]])
    shared["Wl_b"] = np.ascontiguousarray(Wl[64:])
    bl = np.asarray(inputs["bl"], np.float32)
    blT = bl.reshape(8, 128).T  # [128, 8]
    shared["blT2"] = np.repeat(blT, 2, axis=1)  # col = cb*2 + cloud

    Wm1 = np.asarray(inputs["Wm1"], np.float32)
    shared["Wm1r"] = np.ascontiguousarray(
        Wm1.reshape(8, 128, 512).transpose(1, 0, 2).reshape(128, 8 * 512))
    shared["bm1b"] = np.asarray(inputs["bm1"], np.float32).reshape(4, 128).T
    Wm2 = np.asarray(inputs["Wm2"], np.float32)
    shared["Wm2r"] = np.ascontiguousarray(
        Wm2.reshape(4, 128, 256).transpose(1, 0, 2).reshape(128, 4 * 256))
    shared["bm2b"] = np.asarray(inputs["bm2"], np.float32).reshape(2, 128).T
    Wm3 = np.asarray(inputs["Wm3"], np.float32)
    shared["Wm3r"] = np.ascontiguousarray(
        Wm3.reshape(2, 128, 40).transpose(1, 0, 2).reshape(128, 2 * 40))
    shared["bm3T"] = np.asarray(inputs["bm3"], np.float32).reshape(40, 1)

    I64 = np.eye(64, dtype=np.float32)
    shared["I64st"] = np.vstack([I64, I64])
    shared["I40"] = np.eye(40, dtype=np.float32)
    shared["iota_i32"] = np.tile(np.arange(N, dtype=np.int32), (128, 1))
    shared["diag2048"] = (65536 * np.eye(128)).astype(np.int32)
    shared["negA5"] = np.full((3, 1), -0.5, np.float32)
    E1pm = np.zeros((3, 5), np.float32)
    E1pm[:, :3] = np.eye(3, dtype=np.float32)
    shared["E1p"] = E1pm
    E2pm = np.zeros((3, 5), np.float32)
    E2pm[:, 3] = -0.5
    E2pm[:, 4] = -0.5
    shared["E2p"] = E2pm
    shared["ones1024"] = np.ones((1, N), np.float32)

    # self-index wrapped const: col = pb*160 + e*8 + b, partition q,
    # value = point id = pb*128 + b*16 + q; replicated to 4 groups of 16.
    wi = np.zeros((16, 8 * K * 8), np.int16)
    for pb in range(8):
        for e in range(K):
            for b in range(8):
                wi[:, pb * 160 + e * 8 + b] = pb * 128 + b * 16 + np.arange(16)
    shared["wrappedI"] = np.tile(wi, (4, 1))
    return shared


def _host_prep(inputs):
    """Per-core input maps for run_bass_kernel_spmd (fallback path)."""
    shared = _host_prep_shared(inputs)
    pos = np.asarray(inputs["pos"], dtype=np.float32)
    per_core = []
    for core in range(NCORES):
        m = dict(shared)
        m["posT2"] = np.ascontiguousarray(
            pos[CPC * core:CPC * (core + 1)].transpose(0, 2, 1))
        per_core.append(m)
    return per_core


@lru_cache(maxsize=1)
def _get_program():
    return build_program()


# ---------------------------------------------------------------------------
# Fast SPMD dispatch.
#
# run_bass_kernel_spmd under axon redirects to bass2jax.run_bass_via_pjrt,
# which builds a FRESH jax.jit closure per call (re-trace + re-lower every
# time) and re-ships every per-core input (~39 MB of replicated weights) over
# the tunnel on every invocation.  Steady-state cost: ~800 ms/call, of which
# the actual 8-core NEFF execution is <1 ms.
#
# This runner executes the exact same program through the same
# _bass_exec_p/shard_map lowering, but builds the jitted callable once and
# keeps the weight/constant inputs device-resident across calls (re-uploading
# them only if the caller passes different weights).  Only pos (~196 KB) moves
# per call.  Steady-state cost: the per-execute tunnel round trip (~70 ms).
# ---------------------------------------------------------------------------

_WEIGHT_NAMES = ("W1a", "b1a", "W1b", "b1b", "W1c", "b1c", "W2", "b2",
                 "Wl", "bl", "Wm1", "bm1", "Wm2", "bm2", "Wm3", "bm3")


class _FastRunner:
    def __init__(self, nc):
        import jax
        from jax.sharding import Mesh, PartitionSpec, NamedSharding
        import warnings
        with warnings.catch_warnings():
            warnings.simplefilter("ignore", DeprecationWarning)
            try:
                from jax.experimental.shard_map import shard_map
            except ImportError:
                from jax import shard_map as _sm
                def shard_map(f, *, mesh, in_specs, out_specs, check_rep):
                    return _sm(f, mesh=mesh, in_specs=in_specs,
                               out_specs=out_specs, check_vma=check_rep)
        from concourse.bass2jax import (_bass_exec_p, install_neuronx_cc_hook,
                                        partition_id_tensor)

        self.jax = jax
        self.nc = nc
        install_neuronx_cc_hook()

        pn = nc.partition_id_tensor.name if nc.partition_id_tensor else None
        in_names, out_names, out_avals, zero_shapes = [], [], [], []
        for alloc in nc.m.functions[0].allocations:
            if not isinstance(alloc, mybir.MemoryLocationSet):
                continue
            name = alloc.memorylocations[0].name
            if alloc.kind == "ExternalInput":
                if name != pn:
                    in_names.append(name)
            elif alloc.kind == "ExternalOutput":
                out_names.append(name)
                shape = tuple(alloc.tensor_shape)
                dtype = mybir.dt.np(alloc.dtype)
                out_avals.append(jax.core.ShapedArray(shape, dtype))
                zero_shapes.append((shape, dtype))
        self.in_names, self.out_names = in_names, out_names
        all_in_names = in_names + out_names
        if pn is not None:
            all_in_names.append(pn)

        def _body(*args):
            operands = list(args)
            if pn is not None:
                operands.append(partition_id_tensor())
            return tuple(_bass_exec_p.bind(
                *operands,
                out_avals=tuple(out_avals), in_names=tuple(all_in_names),
                out_names=tuple(out_names), lowering_input_output_aliases=(),
                sim_require_finite=True, sim_require_nnan=True, nc=nc))

        devices = jax.devices()[:NCORES]
        mesh = Mesh(np.asarray(devices), ("core",))
        self.sharding = NamedSharding(mesh, PartitionSpec("core"))
        n_args = len(in_names) + len(out_names)
        self.jit = jax.jit(
            shard_map(_body, mesh=mesh,
                      in_specs=(PartitionSpec("core"),) * n_args,
                      out_specs=(PartitionSpec("core"),) * len(out_names),
                      check_rep=False),
            keep_unused=True)
        # Output operand buffers: the NEFF fully writes out2, so these are
        # never read; without donation they stay valid across calls.
        self.dev_zeros = [
            jax.device_put(np.zeros((NCORES * s[0], *s[1:]), d), self.sharding)
            for s, d in zero_shapes]
        self.weight_fp = None     # raw weight arrays of the cached upload
        self.dev_consts = None    # name -> device array (global [8*rows, cols])

    def _upload_consts(self, inputs):
        """Device-put every non-pos input (identical across cores)."""
        shared = _host_prep_shared(inputs)
        dev = {}
        for name in self.in_names:
            if name == "posT2":
                continue
            a = shared[name]
            dev[name] = self.jax.device_put(
                np.concatenate([a] * NCORES, axis=0), self.sharding)
        self.jax.block_until_ready(list(dev.values()))
        self.dev_consts = dev
        self.weight_fp = {k: np.asarray(inputs[k]) for k in _WEIGHT_NAMES}

    def run(self, inputs):
        jax = self.jax
        if self.weight_fp is None or not all(
                np.array_equal(self.weight_fp[k], np.asarray(inputs[k]))
                for k in _WEIGHT_NAMES):
            self._upload_consts(inputs)
        pos = np.asarray(inputs["pos"], dtype=np.float32)
        pos_t = np.ascontiguousarray(pos.transpose(0, 2, 1)).reshape(
            NCORES * CPC, 3, N)
        args = [pos_t if nm == "posT2" else self.dev_consts[nm]
                for nm in self.in_names]
        out = self.jit(*args, *self.dev_zeros)
        res = np.asarray(out[self.out_names.index("out2")])
        return res.reshape(NCORES * CPC, 40).astype(np.float32)


_RUNNER = None


def kernel(**inputs):
    global _RUNNER
    try:
        from concourse._compat import axon_active
        fast_ok = axon_active()
    except Exception:
        fast_ok = False
    if fast_ok:
        try:
            if _RUNNER is None:
                _RUNNER = _FastRunner(_get_program())
            return _RUNNER.run(inputs)
        except Exception as e:
            print(f"kernel: fast path failed ({type(e).__name__}: {e}); "
                  f"falling back to run_bass_kernel_spmd", file=sys.stderr)
            _RUNNER = None
    nc = _get_program()
    in_maps = _host_prep(inputs)
    res = run_bass_kernel_spmd(nc, in_maps, core_ids=list(range(NCORES)))
    outs = [res.results[i]["out2"] for i in range(NCORES)]
    return np.concatenate(outs, axis=0).astype(np.float32)


if __name__ == "__main__":
    pass



# revision 7
# speedup vs baseline: 38.9812x; 1.0866x over previous
"""DGCNN-style point-cloud classifier on 8 Trainium2 NeuronCores.

Data-parallel over the B=16 point-cloud axis: each of the 8 cores processes 2
clouds end-to-end (kNN -> EdgeConv1 -> kNN -> EdgeConv2 -> lin1 -> global max
pool -> head -> log_softmax) with no collectives.  The host only reshapes
inputs/weights and concatenates the 8 per-core [2, 40] outputs.

Key device-side ideas:
  * kNN top-20 per point via packed int32 keys (2^30 - d*S | neighbor index in
    the low 10 bits) extracted with DVE Max8 + MatchReplace (3+2 passes).
  * Neighbor gathers with GPSIMD ap_gather in a feature-major layout, which is
    exactly the transposed layout TensorE wants for the per-edge MLP.
  * EdgeConv2's single linear layer folds through the max-aggregation:
    out_i = pre_i + max_j q_j, so no per-edge GEMM at all.
"""

import sys
import numpy as np
from functools import lru_cache

for _p in ("/opt/trn_rl_repo", "/root/.axon_site/_ro/trn_rl_repo"):
    if _p not in sys.path:
        sys.path.insert(0, _p)

import concourse.bass as bass
import concourse.bacc as bacc
import concourse.mybir as mybir
import concourse.tile as tile
from concourse.bass_utils import run_bass_kernel_spmd

AF = mybir.ActivationFunctionType
ALU = mybir.AluOpType
DT = mybir.dt
F32 = DT.float32
F32R = DT.float32r
I32 = DT.int32
I16 = DT.int16
F16 = DT.float16

N = 1024          # points per cloud
K = 20            # neighbors
NCORES = 8
CPC = 2           # clouds per core
NB = 8            # point blocks of 128 per cloud
E = K * 128       # edges per point block (2560)
NCH = 5           # 512-col chunks per point block of edges

SCALE1 = float(1 << 24)   # key scale for kNN1 (d range 127, resolution 2^-14)
SCALE2 = float(1 << 20)   # key scale for kNN2 (d range 2040, resolution 2^-10)
BIAS30 = float(1 << 30)

# All per-core constants (weights + tables) ride in ONE dram tensor packF
# [128, _PACK_COLS] f32 — int tables are bit-punned into f32 — so each NEFF
# execute binds 4 buffers instead of 31.  (name, rows, cols, dtype) in pack
# order; offsets accumulate.
_PACK_SPEC = [
    ("AmB", 3, 64, F32), ("B3", 3, 64, F32), ("b1a_c", 64, 1, F32),
    ("W1bb", 128, 128, F32), ("b1bb", 128, 1, F32),
    ("W1cc", 128, 128, F32), ("b1cc", 128, 1, F32),
    ("E1r", 128, 66, F32), ("E2r", 128, 66, F32),
    ("W2r2", 128, 128, F32), ("PmQ2", 128, 128, F32), ("b2c", 128, 1, F32),
    ("Wl_a2", 128, 1024, F32), ("Wl_b", 128, 1024, F32), ("blT2", 128, 16, F32),
    ("Wm1r", 128, 4096, F32), ("bm1b", 128, 4, F32),
    ("Wm2r", 128, 1024, F32), ("bm2b", 128, 2, F32),
    ("Wm3r", 128, 80, F32), ("bm3T", 40, 1, F32),
    ("I64st", 128, 64, F32), ("I40", 40, 40, F32),
    ("E1p", 3, 5, F32), ("E2p", 3, 5, F32), ("ones1024", 1, 1024, F32),
    ("iota_i32", 128, 1024, I32), ("diag2048", 128, 128, I32),
]
_PACK_OFF = {}
_PACK_COLS = 0
for _nm, _r, _c, _dt in _PACK_SPEC:
    _PACK_OFF[_nm] = _PACK_COLS
    _PACK_COLS += _c


def _knn_block(nc, pool, psum_alloc, lhsT_A, rhs_B, scale, iota2d, diag2048,
               idx16_all, blk, key_tap=None):
    """Top-20 neighbor indices for one 128-point block.

    lhsT_A: [Kc x 128] block slice of the augmented A operand.
    rhs_B:  [Kc x 1024] augmented B operand. psum = A.T@B = -d/2 per pair.
    Writes int16 indices into idx16_all[:, 20*blk : 20*(blk+1)].
    """
    ps = psum_alloc()
    nc.tensor.matmul(out=ps[:, 0:512], lhsT=lhsT_A,
                     rhs=rhs_B[:, 0:512], start=True, stop=True)
    nc.tensor.matmul(out=ps[:, 512:1024], lhsT=lhsT_A,
                     rhs=rhs_B[:, 512:1024], start=True, stop=True)
    keys = pool.tile([128, N], I32, tag="keys", name="keys")
    nc.scalar.activation(keys[:], ps[:], AF.Copy, bias=BIAS30, scale=scale)
    # clear low 10 bits, boost the diagonal (self) above everything, add index
    nc.vector.tensor_scalar(out=keys[:], in0=keys[:], scalar1=-1024,
                            scalar2=None, op0=ALU.bitwise_and)
    nc.vector.tensor_tensor(out=keys[:, 128 * blk:128 * (blk + 1)],
                            in0=keys[:, 128 * blk:128 * (blk + 1)],
                            in1=diag2048[:], op=ALU.add)
    nc.vector.tensor_tensor(out=keys[:], in0=keys[:], in1=iota2d[:],
                            op=ALU.bitwise_or)
    if key_tap is not None:
        nc.sync.dma_start(out=key_tap, in_=keys[:])
    kf = keys[:].bitcast(F32)
    top = pool.tile([128, 24], F32, tag="top24", name="top24")
    nc.vector.max(out=top[:, 0:8], in_=kf)
    nc.vector.match_replace(out=kf, in_to_replace=top[:, 0:8], in_values=kf,
                            imm_value=0.0)
    nc.vector.max(out=top[:, 8:16], in_=kf)
    nc.vector.match_replace(out=kf, in_to_replace=top[:, 8:16], in_values=kf,
                            imm_value=0.0)
    nc.vector.max(out=top[:, 16:24], in_=kf)
    # col 0 is self; neighbor indices are the low 10 bits of cols 1..20
    idxs = pool.tile([128, K], I32, tag="idx32", name="idx32")
    nc.vector.tensor_scalar(out=idxs[:], in0=top[:, 1:21].bitcast(I32),
                            scalar1=1023, scalar2=None, op0=ALU.bitwise_and)
    nc.vector.tensor_copy(out=idx16_all[:, K * blk:K * (blk + 1)], in_=idxs[:])


def _fold_idx(nc, idx16_all, wrapped, ngroups_log2):
    """[128 x 160] per-point indices -> ap_gather wrapped layout [16 x 1280],
    then replicate across partition groups by doubling."""
    for b in range(8):
        src = idx16_all[16 * b:16 * (b + 1), :].rearrange("q (pb e) -> q pb e", e=K)
        dst = wrapped[0:16, :].rearrange("q (pb e b) -> q pb e b", e=K, b=8)[:, :, :, b]
        nc.sync.dma_start(out=dst, in_=src)
    for i in range(ngroups_log2):
        w = 16 << i
        nc.sync.dma_start(out=wrapped[w:2 * w, :], in_=wrapped[0:w, :])


def build_program(debug_taps=False):
    nc = bacc.Bacc("TRN2", target_bir_lowering=False, debug=False)

    def inp(name, shape, dtype=F32):
        return nc.dram_tensor(name, list(shape), dtype, kind="ExternalInput").ap()

    posT2 = inp("posT2", (CPC, 3, N), F16)
    packF = inp("packF", (128, _PACK_COLS))
    wrappedI = inp("wrappedI", (64, 8 * K * 8), I16)

    out2 = nc.dram_tensor("out2", [CPC, 40], F32, kind="ExternalOutput").ap()
    taps = None
    if debug_taps:
        taps = {
            "dbg_idx1_c0": nc.dram_tensor("dbg_idx1_c0", [128, NB * K], I16,
                                          kind="ExternalOutput").ap(),
            "dbg_keysafter_c0b0": nc.dram_tensor("dbg_keysafter_c0b0", [128, N], I32,
                                                 kind="ExternalOutput").ap(),
            "dbg_x1T": nc.dram_tensor("dbg_x1T", [128, N], F32,
                                      kind="ExternalOutput").ap(),
            "dbg_idx2_c0": nc.dram_tensor("dbg_idx2_c0", [128, NB * K], I16,
                                          kind="ExternalOutput").ap(),
            "dbg_x2T0": nc.dram_tensor("dbg_x2T0", [128, N], F32,
                                       kind="ExternalOutput").ap(),
            "dbg_g2": nc.dram_tensor("dbg_g2", [128, 16], F32,
                                     kind="ExternalOutput").ap(),
            "dbg_G0b0": nc.dram_tensor("dbg_G0b0", [128, E], F32,
                                       kind="ExternalOutput").ap(),
            "dbg_vu0": nc.dram_tensor("dbg_vu0", [128, N], F32,
                                      kind="ExternalOutput").ap(),
        }

    with tile.TileContext(nc) as tc:
        _core_body(tc, posT2, packF, wrappedI, out2, taps)
    nc.compile()
    return nc


def _core_body(tc, posT2, packF, wrappedI, out2, taps=None):
    nc = tc.nc
    from contextlib import ExitStack
    with ExitStack() as ctx:
        cpool = ctx.enter_context(tc.tile_pool(name="consts", bufs=1))
        work = ctx.enter_context(tc.tile_pool(name="work", bufs=3))
        big = ctx.enter_context(tc.tile_pool(name="big", bufs=1))
        persist = ctx.enter_context(tc.tile_pool(name="persist", bufs=1))
        pp = ctx.enter_context(tc.tile_pool(name="ps", bufs=1, space="PSUM"))

        def ps512(shape=None):
            return pp.tile(shape or [128, 512], F32, tag="ps512", name="ps512",
                           bufs=4, padded_shape=[128, 512])

        def ps1024(shape=None):
            return pp.tile(shape or [128, N], F32, tag="ps1024", name="ps1024",
                           bufs=2, padded_shape=[128, N])

        _pack_dims = {nm: (r, c, dt) for nm, r, c, dt in _PACK_SPEC}

        def load_const(name):
            rows, cols, dtype = _pack_dims[name]
            off = _PACK_OFF[name]
            t = cpool.tile([rows, cols], dtype, tag=name, name=f"c_{name}")
            src = packF[0:rows, off:off + cols]
            if dtype != F32:
                src = src.bitcast(dtype)
            nc.sync.dma_start(out=t[:], in_=src)
            return t

        AmB_s = load_const("AmB")
        B3_s = load_const("B3")
        b1a_s = load_const("b1a_c")
        W1bb_s = load_const("W1bb")
        b1bb_s = load_const("b1bb")
        W1cc_s = load_const("W1cc")
        b1cc_s = load_const("b1cc")
        E1r_s = load_const("E1r")
        E2r_s = load_const("E2r")
        W2r2_s = load_const("W2r2")
        PmQ2_s = load_const("PmQ2")
        b2c_s = load_const("b2c")
        Wl_a2_s = load_const("Wl_a2")
        Wl_b_s = load_const("Wl_b")
        blT2_s = load_const("blT2")
        Wm1r_s = load_const("Wm1r")
        bm1b_s = load_const("bm1b")
        Wm2r_s = load_const("Wm2r")
        bm2b_s = load_const("bm2b")
        Wm3r_s = load_const("Wm3r")
        bm3T_s = load_const("bm3T")
        I64st_s = load_const("I64st")
        I40_s = load_const("I40")
        iota_s = load_const("iota_i32")
        diag_s = load_const("diag2048")
        E1p_s = load_const("E1p")
        E2p_s = load_const("E2p")
        _o1 = _PACK_OFF["ones1024"]
        ones1024 = packF[0:1, _o1:_o1 + N]

        # ---------------- Stage A: pos prep per cloud ----------------
        # tag-sharing plan (persist pool, bufs=1 per tag):
        #   ptab{c}: posT -> preT          aug{c}: A5 -> A66
        #   bug{c}:  B5 -> B66             gtab{c}: vu -> qT
        #   wr{c}:   wrapped1 -> wrapped2  xbuf: x1T -> x1sq -> x2T0
        #   xbuf2: x2T1                    x1Tb: alive to lin1
        posT = [persist.tile([3, N], F32, tag=f"ptab{c}", name=f"posT{c}",
                             padded_shape=[128, N]) for c in range(CPC)]
        A5 = [persist.tile([5, N], F32, tag=f"aug{c}", name=f"A5{c}",
                           padded_shape=[128, N]) for c in range(CPC)]
        B5 = [persist.tile([5, N], F32, tag=f"bug{c}", name=f"B5{c}",
                           padded_shape=[128, N]) for c in range(CPC)]
        for c in range(CPC):
            p16 = work.tile([3, N], F16, tag="p16", name="p16")
            nc.sync.dma_start(out=p16[:], in_=posT2[c])
            nc.scalar.activation(posT[c][:], p16[:], AF.Copy)
            p2 = work.tile([3, N], F32, tag="p2", name="p2")
            nc.scalar.activation(p2[:], posT[c][:], AF.Square)
            for h in range(2):
                sl = slice(512 * h, 512 * (h + 1))
                ps5 = ps512([5, 512])
                nc.tensor.matmul(out=ps5[:], lhsT=E1p_s[:],
                                 rhs=posT[c][:, sl],
                                 start=True, stop=False)
                nc.tensor.matmul(out=ps5[:], lhsT=E2p_s[:],
                                 rhs=p2[:, sl],
                                 start=False, stop=True)
                nc.scalar.activation(A5[c][:, sl], ps5[:], AF.Copy)
                nc.scalar.activation(B5[c][:, sl], ps5[:], AF.Copy)
            nc.sync.dma_start(out=A5[c][4:5, :], in_=ones1024)
            nc.sync.dma_start(out=B5[c][3:4, :], in_=ones1024)

        # vu tables: rows 0-63 = v^T = (x@B)^T ; rows 64-127 = u^T = (x@(A-B)+b1a)^T
        vu = [persist.tile([128, N], F32, tag=f"gtab{c}", name=f"vu{c}")
              for c in range(CPC)]
        for c in range(CPC):
            for h in range(2):
                sl = slice(512 * h, 512 * (h + 1))
                pv = ps512([64, 512])
                nc.tensor.matmul(out=pv[:], lhsT=B3_s[:],
                                 rhs=posT[c][:, sl], start=True, stop=True)
                nc.scalar.activation(vu[c][0:64, sl], pv[:], AF.Copy)
                pu = ps512([64, 512])
                nc.tensor.matmul(out=pu[:], lhsT=AmB_s[:],
                                 rhs=posT[c][:, sl], start=True, stop=True)
                nc.scalar.activation(vu[c][64:128, sl], pu[:], AF.Identity,
                                     bias=b1a_s[:])

        if taps is not None:
            nc.sync.dma_start(out=taps["dbg_vu0"], in_=vu[0][:])
        # ---------------- Stage B: kNN1 + fold ----------------
        wrapped1 = [persist.tile([128, 8 * K * 8], I16, tag=f"wr{c}",
                                 name=f"wr1{c}") for c in range(CPC)]
        for c in range(CPC):
            idx16_all = work.tile([128, NB * K], I16, tag="idx16", name="idx16")
            for blk in range(NB):
                _knn_block(nc, work, ps1024, A5[c][:, 128 * blk:128 * (blk + 1)],
                           B5[c][:], SCALE1, iota_s, diag_s, idx16_all, blk,
                           key_tap=(taps["dbg_keysafter_c0b0"]
                                    if taps is not None and c == 0 and blk == 0
                                    else None))
            _fold_idx(nc, idx16_all, wrapped1[c], 2)
            nc.sync.dma_start(out=wrapped1[c][64:128, :], in_=wrappedI)
            if taps is not None and c == 0:
                nc.sync.dma_start(out=taps["dbg_idx1_c0"], in_=idx16_all[:])

        # ---------------- Stage D: conv1 ----------------
        x1T = persist.tile([128, N], F32, tag="xbuf", name="x1T")
        for blk in range(NB):
            G = [None, None]
            for c in range(CPC):
                G[c] = big.tile([128, E], F32, tag="gath", name=f"G{c}", bufs=3)
                nc.gpsimd.ap_gather(
                    out_ap=G[c][:], in_ap=vu[c][:],
                    idxs_ap=wrapped1[c][:, 160 * blk:160 * (blk + 1)],
                    channels=128, num_elems=N, d=1, num_idxs=E)
            if taps is not None and blk == 0:
                nc.sync.dma_start(out=taps["dbg_G0b0"], in_=G[0][:])
            L3 = big.tile([128, E], F32, tag="L3", name="L3", bufs=2)
            for ch in range(NCH):
                sl = slice(512 * ch, 512 * (ch + 1))
                L12 = work.tile([128, 512], F32, tag="L12", name="L12")
                for c in range(CPC):
                    ph = ps512([64, 512])
                    nc.tensor.matmul(out=ph[:], lhsT=I64st_s[:],
                                     rhs=G[c][:, sl],
                                     start=True, stop=True)
                    nc.scalar.activation(L12[64 * c:64 * (c + 1), :], ph[:],
                                         AF.Relu)
                p2l = ps512()
                nc.tensor.matmul(out=p2l[:], lhsT=W1bb_s[:],
                                 rhs=L12[:], start=True, stop=True)
                L2 = work.tile([128, 512], F32, tag="L2", name="L2")
                nc.scalar.activation(L2[:], p2l[:], AF.Relu, bias=b1bb_s[:])
                p3l = ps512()
                nc.tensor.matmul(out=p3l[:], lhsT=W1cc_s[:],
                                 rhs=L2[:], start=True, stop=True)
                nc.scalar.activation(L3[:, sl], p3l[:], AF.Copy)
            nc.vector.tensor_reduce(
                out=x1T[:, 128 * blk:128 * (blk + 1)],
                in_=L3[:].rearrange("c (e p) -> c p e", p=128),
                axis=mybir.AxisListType.X, op=ALU.max)
        x1Tb = persist.tile([128, N], F32, tag="x1Tb", name="x1Tb")
        nc.scalar.activation(x1Tb[:], x1T[:], AF.Identity, bias=b1cc_s[:])
        if taps is not None:
            nc.sync.dma_start(out=taps["dbg_x1T"], in_=x1Tb[:])

        # ---------------- Stage E: kNN2 + fold ----------------
        x1sq = persist.tile([128, N], F32, tag="xbuf", name="x1sq")
        nc.scalar.activation(x1sq[:], x1Tb[:], AF.Square)
        A66 = [persist.tile([66, N], F32, tag=f"aug{c}", name=f"A66{c}",
                            padded_shape=[128, N]) for c in range(CPC)]
        B66 = [persist.tile([66, N], F32, tag=f"bug{c}", name=f"B66{c}",
                            padded_shape=[128, N]) for c in range(CPC)]
        for c in range(CPC):
            half = slice(64 * c, 64 * (c + 1))
            for h in range(2):
                sl = slice(512 * h, 512 * (h + 1))
                p66 = ps512([66, 512])
                nc.tensor.matmul(out=p66[:], lhsT=E1r_s[half, :],
                                 rhs=x1Tb[half, sl],
                                 start=True, stop=False)
                nc.tensor.matmul(out=p66[:], lhsT=E2r_s[half, :],
                                 rhs=x1sq[half, sl],
                                 start=False, stop=True)
                nc.scalar.activation(A66[c][:, sl], p66[:], AF.Copy)
                nc.scalar.activation(B66[c][:, sl], p66[:], AF.Copy)
            nc.sync.dma_start(out=A66[c][65:66, :], in_=ones1024)
            nc.sync.dma_start(out=B66[c][64:65, :], in_=ones1024)

        wrapped2 = [persist.tile([128, 8 * K * 8], I16, tag=f"wr{c}",
                                 name=f"wr2{c}") for c in range(CPC)]
        for c in range(CPC):
            idx16_all = work.tile([128, NB * K], I16, tag="idx16", name="idx16")
            for blk in range(NB):
                _knn_block(nc, work, ps1024, A66[c][:, 128 * blk:128 * (blk + 1)],
                           B66[c][:], SCALE2, iota_s, diag_s, idx16_all, blk)
            _fold_idx(nc, idx16_all, wrapped2[c], 3)
            if taps is not None and c == 0:
                nc.sync.dma_start(out=taps["dbg_idx2_c0"], in_=idx16_all[:])

        # ---------------- Stage F: conv2 ----------------
        x2T = [persist.tile([128, N], F32, tag=("xbuf" if c == 0 else "xbuf2"),
                            name=f"x2T{c}") for c in range(CPC)]
        qT = [persist.tile([128, N], F32, tag=f"gtab{c}", name=f"qT{c}")
              for c in range(CPC)]
        preT = [persist.tile([128, N], F32, tag=f"ptab{c}", name=f"preT{c}")
                for c in range(CPC)]
        for c in range(CPC):
            half = slice(64 * c, 64 * (c + 1))
            for h in range(2):
                sl = slice(512 * h, 512 * (h + 1))
                pq = ps512()
                nc.tensor.matmul(out=pq[:], lhsT=W2r2_s[half, :],
                                 rhs=x1Tb[half, sl], start=True, stop=True)
                nc.scalar.activation(qT[c][:, sl], pq[:], AF.Copy)
                ppre = ps512()
                nc.tensor.matmul(out=ppre[:], lhsT=PmQ2_s[half, :],
                                 rhs=x1Tb[half, sl], start=True, stop=True)
                nc.scalar.activation(preT[c][:, sl], ppre[:], AF.Identity,
                                     bias=b2c_s[:])
            for blk in range(NB):
                Gq = big.tile([128, E], F32, tag="gath", name="Gq", bufs=3)
                nc.gpsimd.ap_gather(
                    out_ap=Gq[:], in_ap=qT[c][:],
                    idxs_ap=wrapped2[c][:, 160 * blk:160 * (blk + 1)],
                    channels=128, num_elems=N, d=1, num_idxs=E)
                red = work.tile([128, 128], F32, tag="red", name="red")
                nc.vector.tensor_reduce(
                    out=red[:], in_=Gq[:].rearrange("c (e p) -> c p e", p=128),
                    axis=mybir.AxisListType.X, op=ALU.max)
                nc.vector.tensor_tensor(
                    out=x2T[c][:, 128 * blk:128 * (blk + 1)], in0=red[:],
                    in1=preT[c][:, 128 * blk:128 * (blk + 1)], op=ALU.add)

        if taps is not None:
            nc.sync.dma_start(out=taps["dbg_x2T0"], in_=x2T[0][:])
        # ---------------- Stage G: lin1 + global max pool ----------------
        g2 = persist.tile([128, 16], F32, tag="g2", name="g2")
        for c in range(CPC):
            half = slice(64 * c, 64 * (c + 1))
            for cb in range(8):
                cbs = slice(128 * cb, 128 * (cb + 1))
                pl = ps1024()
                for h in range(2):
                    sl = slice(512 * h, 512 * (h + 1))
                    nc.tensor.matmul(out=pl[:, sl],
                                     lhsT=Wl_a2_s[half, cbs],
                                     rhs=x1Tb[half, sl],
                                     start=True, stop=False)
                    nc.tensor.matmul(out=pl[:, sl],
                                     lhsT=Wl_b_s[:, cbs],
                                     rhs=x2T[c][:, sl],
                                     start=False, stop=True)
                nc.vector.tensor_reduce(out=g2[:, 2 * cb + c:2 * cb + c + 1],
                                        in_=pl[:], axis=mybir.AxisListType.X,
                                        op=ALU.max)
        nc.vector.tensor_tensor(out=g2[:], in0=g2[:], in1=blT2_s[:], op=ALU.add)
        if taps is not None:
            nc.sync.dma_start(out=taps["dbg_g2"], in_=g2[:])

        # ---------------- Stage H: head + log_softmax ----------------
        h1s = persist.tile([128, 8], F32, tag="h1s", name="h1s")
        for m in range(4):
            ph = ps512([128, 2])
            for k in range(8):
                nc.tensor.matmul(out=ph[:],
                                 lhsT=Wm1r_s[:, 512 * k + 128 * m:512 * k + 128 * (m + 1)],
                                 rhs=g2[:, 2 * k:2 * (k + 1)],
                                 start=(k == 0), stop=(k == 7))
            nc.scalar.activation(h1s[:, 2 * m:2 * (m + 1)], ph[:], AF.Relu,
                                 bias=bm1b_s[:, m:m + 1])
        h2s = persist.tile([128, 4], F32, tag="h2s", name="h2s")
        for m in range(2):
            ph = ps512([128, 2])
            for j in range(4):
                nc.tensor.matmul(out=ph[:],
                                 lhsT=Wm2r_s[:, 256 * j + 128 * m:256 * j + 128 * (m + 1)],
                                 rhs=h1s[:, 2 * j:2 * (j + 1)],
                                 start=(j == 0), stop=(j == 3))
            nc.scalar.activation(h2s[:, 2 * m:2 * (m + 1)], ph[:], AF.Relu,
                                 bias=bm2b_s[:, m:m + 1])
        plg = ps512([40, 2])
        for j in range(2):
            nc.tensor.matmul(out=plg[:], lhsT=Wm3r_s[:, 40 * j:40 * (j + 1)],
                             rhs=h2s[:, 2 * j:2 * (j + 1)],
                             start=(j == 0), stop=(j == 1))
        lg = persist.tile([40, 2], F32, tag="lg", name="lg")
        nc.scalar.activation(lg[:], plg[:], AF.Identity, bias=bm3T_s[:])
        pt = ps512([2, 40])
        nc.tensor.transpose(out=pt[:], in_=lg[:], identity=I40_s[:])
        lgT = persist.tile([2, 40], F32, tag="lgT", name="lgT")
        nc.scalar.activation(lgT[:], pt[:], AF.Copy)
        negm = persist.tile([2, 1], F32, tag="negm", name="negm")
        nc.vector.tensor_reduce(out=negm[:], in_=lgT[:],
                                axis=mybir.AxisListType.X, op=ALU.max,
                                negate=True)
        t1 = persist.tile([2, 40], F32, tag="t1", name="t1")
        nc.scalar.activation(t1[:], lgT[:], AF.Identity, bias=negm[:])
        ex = persist.tile([2, 40], F32, tag="ex", name="ex")
        nc.scalar.activation(ex[:], lgT[:], AF.Exp, bias=negm[:])
        ssum = persist.tile([2, 1], F32, tag="ssum", name="ssum")
        nc.vector.tensor_reduce(out=ssum[:], in_=ex[:],
                                axis=mybir.AxisListType.X, op=ALU.add)
        lsum = persist.tile([2, 1], F32, tag="lsum", name="lsum")
        nc.scalar.activation(lsum[:], ssum[:], AF.Ln)
        outt = persist.tile([2, 40], F32, tag="outt", name="outt")
        nc.vector.tensor_tensor(out=outt[:], in0=t1[:],
                                in1=lsum[:].to_broadcast([2, 40]),
                                op=ALU.subtract)
        nc.sync.dma_start(out=out2, in_=outt[:])


def _host_prep_shared(inputs):
    """Build the shared (weight/const) input map — everything except posT2."""
    W1a = np.asarray(inputs["W1a"], np.float32)
    shared = {}
    shared["AmB"] = np.ascontiguousarray(W1a[:3] - W1a[3:])
    shared["B3"] = np.ascontiguousarray(W1a[3:])
    shared["b1a_c"] = np.asarray(inputs["b1a"], np.float32).reshape(64, 1)

    def blockdiag2(w):
        z = np.zeros((128, 128), np.float32)
        z[:64, :64] = w
        z[64:, 64:] = w
        return z

    shared["W1bb"] = blockdiag2(np.asarray(inputs["W1b"], np.float32))
    shared["b1bb"] = np.tile(np.asarray(inputs["b1b"], np.float32), 2).reshape(128, 1)
    shared["W1cc"] = blockdiag2(np.asarray(inputs["W1c"], np.float32))
    shared["b1cc"] = np.tile(np.asarray(inputs["b1c"], np.float32), 2).reshape(128, 1)

    E1 = np.zeros((64, 66), np.float32)
    E1[:, :64] = np.eye(64, dtype=np.float32)
    E2 = np.zeros((64, 66), np.float32)
    E2[:, 64] = -0.5
    E2[:, 65] = -0.5
    shared["E1r"] = np.vstack([E1, E1])
    shared["E2r"] = np.vstack([E2, E2])

    W2 = np.asarray(inputs["W2"], np.float32)
    shared["W2r2"] = np.vstack([W2[64:], W2[64:]])
    shared["PmQ2"] = np.vstack([W2[:64] - W2[64:], W2[:64] - W2[64:]])
    shared["b2c"] = np.asarray(inputs["b2"], np.float32).reshape(128, 1)

    Wl = np.asarray(inputs["Wl"], np.float32)
    shared["Wl_a2"] = np.vstack([Wl[:64], Wl[:64]])
    shared["Wl_b"] = np.ascontiguousarray(Wl[64:])
    bl = np.asarray(inputs["bl"], np.float32)
    blT = bl.reshape(8, 128).T  # [128, 8]
    shared["blT2"] = np.repeat(blT, 2, axis=1)  # col = cb*2 + cloud

    Wm1 = np.asarray(inputs["Wm1"], np.float32)
    shared["Wm1r"] = np.ascontiguousarray(
        Wm1.reshape(8, 128, 512).transpose(1, 0, 2).reshape(128, 8 * 512))
    shared["bm1b"] = np.asarray(inputs["bm1"], np.float32).reshape(4, 128).T
    Wm2 = np.asarray(inputs["Wm2"], np.float32)
    shared["Wm2r"] = np.ascontiguousarray(
        Wm2.reshape(4, 128, 256).transpose(1, 0, 2).reshape(128, 4 * 256))
    shared["bm2b"] = np.asarray(inputs["bm2"], np.float32).reshape(2, 128).T
    Wm3 = np.asarray(inputs["Wm3"], np.float32)
    shared["Wm3r"] = np.ascontiguousarray(
        Wm3.reshape(2, 128, 40).transpose(1, 0, 2).reshape(128, 2 * 40))
    shared["bm3T"] = np.asarray(inputs["bm3"], np.float32).reshape(40, 1)

    I64 = np.eye(64, dtype=np.float32)
    shared["I64st"] = np.vstack([I64, I64])
    shared["I40"] = np.eye(40, dtype=np.float32)
    shared["iota_i32"] = np.tile(np.arange(N, dtype=np.int32), (128, 1))
    shared["diag2048"] = (65536 * np.eye(128)).astype(np.int32)
    E1pm = np.zeros((3, 5), np.float32)
    E1pm[:, :3] = np.eye(3, dtype=np.float32)
    shared["E1p"] = E1pm
    E2pm = np.zeros((3, 5), np.float32)
    E2pm[:, 3] = -0.5
    E2pm[:, 4] = -0.5
    shared["E2p"] = E2pm
    shared["ones1024"] = np.ones((1, N), np.float32)

    # self-index wrapped const: col = pb*160 + e*8 + b, partition q,
    # value = point id = pb*128 + b*16 + q; replicated to 4 groups of 16.
    wi = np.zeros((16, 8 * K * 8), np.int16)
    for pb in range(8):
        for e in range(K):
            for b in range(8):
                wi[:, pb * 160 + e * 8 + b] = pb * 128 + b * 16 + np.arange(16)
    wrappedI = np.tile(wi, (4, 1))

    packF = np.zeros((128, _PACK_COLS), np.float32)
    for nm, rows, cols, dt in _PACK_SPEC:
        a = np.asarray(shared[nm])
        if a.dtype != np.float32:
            a = a.view(np.float32)
        packF[0:rows, _PACK_OFF[nm]:_PACK_OFF[nm] + cols] = a.reshape(rows, cols)
    return {"packF": packF, "wrappedI": wrappedI}


def _host_prep(inputs):
    """Per-core input maps for run_bass_kernel_spmd (fallback path)."""
    shared = _host_prep_shared(inputs)
    pos = np.asarray(inputs["pos"], dtype=np.float32)
    per_core = []
    for core in range(NCORES):
        m = dict(shared)
        m["posT2"] = np.ascontiguousarray(
            pos[CPC * core:CPC * (core + 1)].transpose(0, 2, 1)
            .astype(np.float16))
        per_core.append(m)
    return per_core


@lru_cache(maxsize=1)
def _get_program():
    return build_program()


# ---------------------------------------------------------------------------
# Fast SPMD dispatch.
#
# run_bass_kernel_spmd under axon redirects to bass2jax.run_bass_via_pjrt,
# which builds a FRESH jax.jit closure per call (re-trace + re-lower every
# time) and re-ships every per-core input (~39 MB of replicated weights) over
# the tunnel on every invocation.  Steady-state cost: ~800 ms/call, of which
# the actual 8-core NEFF execution is <1 ms.
#
# This runner executes the exact same program through the same
# _bass_exec_p/shard_map lowering, but builds the jitted callable once and
# keeps the weight/constant inputs device-resident across calls (re-uploading
# them only if the caller passes different weights).  Only pos (~196 KB) moves
# per call.  Steady-state cost: the per-execute tunnel round trip (~70 ms).
# ---------------------------------------------------------------------------

_WEIGHT_NAMES = ("W1a", "b1a", "W1b", "b1b", "W1c", "b1c", "W2", "b2",
                 "Wl", "bl", "Wm1", "bm1", "Wm2", "bm2", "Wm3", "bm3")


class _FastRunner:
    def __init__(self, nc):
        import jax
        from jax.sharding import Mesh, PartitionSpec, NamedSharding
        import warnings
        with warnings.catch_warnings():
            warnings.simplefilter("ignore", DeprecationWarning)
            try:
                from jax.experimental.shard_map import shard_map
            except ImportError:
                from jax import shard_map as _sm
                def shard_map(f, *, mesh, in_specs, out_specs, check_rep):
                    return _sm(f, mesh=mesh, in_specs=in_specs,
                               out_specs=out_specs, check_vma=check_rep)
        from concourse.bass2jax import (_bass_exec_p, install_neuronx_cc_hook,
                                        partition_id_tensor)

        self.jax = jax
        self.nc = nc
        install_neuronx_cc_hook()

        pn = nc.partition_id_tensor.name if nc.partition_id_tensor else None
        in_names, out_names, out_avals, zero_shapes = [], [], [], []
        for alloc in nc.m.functions[0].allocations:
            if not isinstance(alloc, mybir.MemoryLocationSet):
                continue
            name = alloc.memorylocations[0].name
            if alloc.kind == "ExternalInput":
                if name != pn:
                    in_names.append(name)
            elif alloc.kind == "ExternalOutput":
                out_names.append(name)
                shape = tuple(alloc.tensor_shape)
                dtype = mybir.dt.np(alloc.dtype)
                out_avals.append(jax.core.ShapedArray(shape, dtype))
                zero_shapes.append((shape, dtype))
        self.in_names, self.out_names = in_names, out_names
        all_in_names = in_names + out_names
        if pn is not None:
            all_in_names.append(pn)

        def _body(*args):
            operands = list(args)
            if pn is not None:
                operands.append(partition_id_tensor())
            return tuple(_bass_exec_p.bind(
                *operands,
                out_avals=tuple(out_avals), in_names=tuple(all_in_names),
                out_names=tuple(out_names), lowering_input_output_aliases=(),
                sim_require_finite=True, sim_require_nnan=True, nc=nc))

        devices = jax.devices()[:NCORES]
        mesh = Mesh(np.asarray(devices), ("core",))
        self.sharding = NamedSharding(mesh, PartitionSpec("core"))
        n_args = len(in_names) + len(out_names)
        self.jit = jax.jit(
            shard_map(_body, mesh=mesh,
                      in_specs=(PartitionSpec("core"),) * n_args,
                      out_specs=(PartitionSpec("core"),) * len(out_names),
                      check_rep=False),
            keep_unused=True)
        # Output operand buffers: the NEFF fully writes out2, so these are
        # never read; without donation they stay valid across calls.
        self.dev_zeros = [
            jax.device_put(np.zeros((NCORES * s[0], *s[1:]), d), self.sharding)
            for s, d in zero_shapes]
        self.weight_fp = None     # raw weight arrays of the cached upload
        self.dev_consts = None    # name -> device array (global [8*rows, cols])

    def _upload_consts(self, inputs):
        """Device-put every non-pos input (identical across cores)."""
        shared = _host_prep_shared(inputs)
        dev = {}
        for name in self.in_names:
            if name == "posT2":
                continue
            a = shared[name]
            dev[name] = self.jax.device_put(
                np.concatenate([a] * NCORES, axis=0), self.sharding)
        self.jax.block_until_ready(list(dev.values()))
        self.dev_consts = dev
        self.weight_fp = {k: np.asarray(inputs[k]) for k in _WEIGHT_NAMES}

    def run(self, inputs):
        jax = self.jax
        if self.weight_fp is None or not all(
                np.array_equal(self.weight_fp[k], np.asarray(inputs[k]))
                for k in _WEIGHT_NAMES):
            self._upload_consts(inputs)
        pos = np.asarray(inputs["pos"], dtype=np.float32)
        pos_t = np.ascontiguousarray(
            pos.transpose(0, 2, 1).astype(np.float16)).reshape(
            NCORES * CPC, 3, N)
        args = [pos_t if nm == "posT2" else self.dev_consts[nm]
                for nm in self.in_names]
        out = self.jit(*args, *self.dev_zeros)
        res = np.asarray(out[self.out_names.index("out2")])
        return res.reshape(NCORES * CPC, 40).astype(np.float32)


_RUNNER = None


def kernel(**inputs):
    global _RUNNER
    try:
        from concourse._compat import axon_active
        fast_ok = axon_active()
    except Exception:
        fast_ok = False
    if fast_ok:
        try:
            if _RUNNER is None:
                _RUNNER = _FastRunner(_get_program())
            return _RUNNER.run(inputs)
        except Exception as e:
            print(f"kernel: fast path failed ({type(e).__name__}: {e}); "
                  f"falling back to run_bass_kernel_spmd", file=sys.stderr)
            _RUNNER = None
    nc = _get_program()
    in_maps = _host_prep(inputs)
    res = run_bass_kernel_spmd(nc, in_maps, core_ids=list(range(NCORES)))
    outs = [res.results[i]["out2"] for i in range(NCORES)]
    return np.concatenate(outs, axis=0).astype(np.float32)


if __name__ == "__main__":
    pass

